# revision 3
# baseline (speedup 1.0000x reference)
"""Distributed Bass kernel for nn_AdaGNN (2-layer GAT + MLP heads + CE losses).

Strategy (8 NeuronCores, SPMD):
  - Nodes assigned to 8 cores x NT tiles of 128 by a load-balancing packer
    (equal edge counts per tile). Output is permutation invariant.
  - Per layer: dense per-node transform producing a 128-wide (256B) gather-table
    row [feat(64) | a_src(8) | a_dst(8) | pad] bf16 -> AllGather -> per-edge
    dma_gather of SRC rows (int16 indices; 4 source windows of TBL/4 rows) and
    of DST rows from the LOCAL table (per-edge a_dst without any transpose) ->
    segment softmax via exp (value ranges are small; max-subtraction
    unnecessary) -> weighted segment-sum via one-hot matmuls on TensorE ->
    normalize.
  - Edge chunks of 128 are keyed (tile, window, q) with a per-tile/window
    chunk schedule shared by all cores (SPMD-static); tiles are grouped into
    batches of identical schedule vectors; all per-edge tensors are laid out
    window-major so per-batch vector ops are single instructions.
  - Layer 2 aggregates per-head-weighted 64-dim inputs (512-wide messages) and
    applies the reshuffled W2 (mean over heads folded in) after aggregation.
  - Layer-2 table rows are stored at finalize-order positions so the staged
    7-tile row groups write with one DMA each.
  - MLP heads + masked CE per dst tile; partial sums AllReduced; final scalar
    computed on device.
"""

import math
import numpy as np
import ml_dtypes

import concourse.bass as bass
import concourse.tile as tile
from concourse import mybir
from concourse.bacc import Bacc
from concourse.bass_utils import run_bass_kernel_spmd

BF16 = mybir.dt.bfloat16
F32 = mybir.dt.float32
I16 = mybir.dt.int16
P = 128
NCORES = 8
NW = 4          # gather windows
RW = 128        # table row width (elements, bf16) = 256B
AF = mybir.ActivationFunctionType
OP = mybir.AluOpType

nbf = ml_dtypes.bfloat16


# ----------------------------------------------------------------------------
# Host-side graph preprocessing
# ----------------------------------------------------------------------------

def _wcat2(tsw2, clsw2):
    w = np.zeros((128, 8), np.float32)
    w[0:64, 0:5] = tsw2
    w[64:128, 5:7] = clsw2
    return w


def _prep(inputs, tiles_per_batch=7):
    x = np.asarray(inputs["x"], np.float32)
    ei = np.asarray(inputs["edge_index"], np.int32)
    N, D_IN = x.shape
    NPC = N // NCORES
    NT = math.ceil(NPC / P)
    NPAD = NT * P
    TBL = NCORES * NPAD
    WIN = TBL // NW
    NBINS = NCORES * NT

    # self-loops are handled analytically on-device (diagonal term of the
    # segment softmax); only the regular edges go through the gather path
    src = ei[0]
    dst = ei[1]

    # ---- balanced node -> (core, tile, slot) assignment ----
    import heapq
    deg = np.bincount(dst, minlength=N).astype(np.int64)
    order_n = np.argsort(-deg, kind="stable")
    heap = [(0, b) for b in range(NBINS)]
    heapq.heapify(heap)
    bin_cnt = np.zeros(NBINS, np.int64)
    bin_edges = np.zeros(NBINS, np.int64)
    node_bin = np.zeros(N, np.int32)
    node_slot = np.zeros(N, np.int32)
    for n in order_n:
        while True:
            e, b = heapq.heappop(heap)
            if e == bin_edges[b] and bin_cnt[b] < P:
                break
        node_bin[n] = b
        node_slot[n] = bin_cnt[b]
        bin_cnt[b] += 1
        bin_edges[b] += deg[n]
        if bin_cnt[b] < P:
            heapq.heappush(heap, (int(bin_edges[b]), b))
    node_core = node_bin // NT
    node_tile = node_bin % NT

    rowpos = node_core.astype(np.int64) * NPAD + node_tile * P + node_slot

    core_of = node_core[dst]
    tile_of = node_tile[dst]
    loc_of = node_slot[dst]
    srow = rowpos[src]
    win_of = (srow // WIN).astype(np.int32)

    # per (core, tile, window) counts -> shared schedule
    cnt = np.zeros((NCORES, NT, NW), np.int64)
    np.add.at(cnt, (core_of, tile_of, win_of), 1)
    chs = np.ceil(cnt / P).astype(np.int64).max(axis=0)  # [NT, NW]
    chs[:, 0] = np.maximum(1, chs[:, 0])  # every tile aggregates >= 1 chunk

    # group tiles by schedule vector; build batches of identical structure
    keys = [tuple(chs[t]) for t in range(NT)]
    order_t = sorted(range(NT), key=lambda t: (keys[t], t))
    batches = []  # (tiles, cvec)
    i = 0
    while i < NT:
        j = i
        S_i = int(sum(keys[order_t[i]]))
        while (j < NT and keys[order_t[j]] == keys[order_t[i]]
               and j - i < tiles_per_batch
               and (j - i + 1) * S_i <= 64):
            j += 1
        batches.append(([order_t[k] for k in range(i, j)],
                        np.array(keys[order_t[i]], np.int64)))
        i = j

    # chunk bookkeeping in batch order
    CH = 0
    CHW = np.zeros(NW, np.int64)
    binfo = []  # (c0, cw0[4], tiles, cvec)
    for tiles, cvec in batches:
        binfo.append((CH, CHW.copy(), tiles, cvec))
        CH += int(cvec.sum()) * len(tiles)
        CHW += cvec * len(tiles)
    CH = int(CH)

    # finalize order (the order edge_layer visits tiles, batch-major) and the
    # layer-2 table row permutation: tbl2 rows live at finalize positions
    fo = np.array([t for (_, _, tiles, _) in binfo for t in tiles], np.int64)
    fp = np.zeros(NT, np.int64)
    fp[fo] = np.arange(NT)
    rowpos2 = node_core.astype(np.int64) * NPAD + fp[node_tile] * P + node_slot
    srow2 = rowpos2[src]

    # per-core edge arrays (chunk positions are WINDOW-MAJOR inside batches)
    per_core = []
    for c in range(NCORES):
        sel = core_of == c
        s_row, s_row2 = srow[sel], srow2[sel]
        t_c, l_c, w_c = tile_of[sel], loc_of[sel], win_of[sel]
        srcw = [np.zeros(max(1, int(CHW[w])) * P, np.int16) for w in range(NW)]
        srcw2 = [np.zeros(max(1, int(CHW[w])) * P, np.int16) for w in range(NW)]
        dstloc = np.full((CH, P), -1.0, np.float32)
        dsti = np.zeros((CH, P), np.int16)   # layer-1 local dst row (tile-id based)
        dsti2 = np.zeros((CH, P), np.int16)  # layer-2 local dst row (fin-pos based)
        for (c0, cw0, tiles, cvec) in binfo:
            nb = len(tiles)
            woff = []
            o = 0
            for w in range(NW):
                woff.append(o)
                o += nb * int(cvec[w])
            for i_t, t in enumerate(tiles):
                for w in range(NW):
                    cw = int(cvec[w])
                    if cw == 0:
                        continue
                    m = (t_c == t) & (w_c == w)
                    k = int(m.sum())
                    assert k <= cw * P, (k, cw)
                    rows_l = (s_row[m] - w * WIN).astype(np.int16)
                    rows_l2 = (s_row2[m] - w * WIN).astype(np.int16)
                    lt = l_c[m]
                    gp = c0 + woff[w] + i_t * cw       # window-major position
                    wp = int(cw0[w]) + i_t * cw        # window-local position
                    j = np.arange(k)
                    srcw[w][(wp + j // P) * P + (j % P)] = rows_l
                    srcw2[w][(wp + j // P) * P + (j % P)] = rows_l2
                    dstloc[gp + j // P, j % P] = lt
                    dsti[gp + j // P, j % P] = t * P + lt
                    dsti2[gp + j // P, j % P] = fp[t] * P + lt

        def wrap(ids):
            a = ids.reshape(-1, 16).T.copy()
            return np.tile(a, (8, 1)).astype(np.int16)

        per_core.append((
            [wrap(srcw[w]) for w in range(NW)],
            [wrap(srcw2[w]) for w in range(NW)],
            dstloc.T.copy(),
            wrap(dsti.reshape(-1)),
            wrap(dsti2.reshape(-1)),
        ))

    # ----- weights / constants (replicated) -----
    f32 = np.float32
    W1 = np.asarray(inputs["W1"], f32)
    as1 = np.asarray(inputs["att_src1"], f32)
    ad1 = np.asarray(inputs["att_dst1"], f32)
    W1h = W1.reshape(D_IN, 8, 8)
    wtab1 = np.concatenate(
        [W1, np.einsum("khc,hc->kh", W1h, as1), np.einsum("khc,hc->kh", W1h, ad1)], 1
    )  # [D_IN, 80]
    KA = 128 if D_IN > 128 else D_IN
    KB = D_IN - KA

    W2 = np.asarray(inputs["W2"], f32)
    as2 = np.asarray(inputs["att_src2"], f32)
    ad2 = np.asarray(inputs["att_dst2"], f32)
    W2h = W2.reshape(64, 8, 64)
    wsd2 = np.concatenate(
        [np.einsum("khc,hc->kh", W2h, as2), np.einsum("khc,hc->kh", W2h, ad2)], 1
    )  # [64, 16]
    wbig = (W2h.transpose(1, 0, 2).reshape(512, 64) / 8.0)
    wbig_dev = wbig.reshape(4, 128, 64).transpose(1, 0, 2).reshape(128, 256)

    consts = {
        "wtab1": wtab1.astype(nbf),
        "wsd2": wsd2.astype(nbf),
        "wbig": wbig_dev.astype(nbf),
        "w1cat": np.concatenate([np.asarray(inputs["ts_w1"], f32),
                                 np.asarray(inputs["cls_w1"], f32)], 1).astype(nbf),
        "b1cat": np.concatenate([np.asarray(inputs["ts_b1"], f32),
                                 np.asarray(inputs["cls_b1"], f32)]).reshape(P, 1),
        "wcat2": _wcat2(np.asarray(inputs["ts_w2"], f32),
                        np.asarray(inputs["cls_w2"], f32)).astype(nbf),
        "bcat2": np.concatenate([np.asarray(inputs["ts_b2"], f32),
                                 np.asarray(inputs["cls_b2"], f32),
                                 np.zeros(1, f32)]).reshape(8, 1),
        "b1r": np.tile(np.asarray(inputs["b1"], f32)[None, :], (P, 1)),
        "b2r": np.tile(np.asarray(inputs["b2"], f32)[None, :], (P, 1)),
        "iota": np.tile(np.arange(P, dtype=f32)[None, :], (P, 1)).astype(nbf),
        "ident": np.eye(P, dtype=f32).astype(nbf),
        "identf": np.eye(P, dtype=f32),
        "ones": np.ones((P, 1), f32),
    }

    tst = np.asarray(inputs["timestamp_target"], np.int64)
    clt = np.asarray(inputs["node_target"], np.int64)
    msk = np.asarray(inputs["node_mask"]).astype(f32)

    in_maps = []
    pos_in_core = node_tile.astype(np.int64) * P + node_slot
    for c in range(NCORES):
        srcw, srcw2, dstloc, dsti, dsti2 = per_core[c]
        mine = np.nonzero(node_core == c)[0]
        pos = pos_in_core[mine]
        xT = np.zeros((D_IN, NPAD), f32)
        xT[:, pos] = x[mine].T
        valid = np.zeros(NPAD, bool)
        valid[pos] = True
        g_ts = np.zeros(NPAD, np.int64)
        g_ts[pos] = tst[mine]
        g_cl = np.zeros(NPAD, np.int64)
        g_cl[pos] = clt[mine]
        g_mk = np.zeros(NPAD, f32)
        g_mk[pos] = msk[mine]
        rows = np.arange(NPAD)
        ohts = np.zeros((NPAD, 5), f32)
        ohts[rows, g_ts] = 1.0
        ohcl = np.zeros((NPAD, 2), f32)
        ohcl[rows, g_cl] = 1.0

        def pmf(a, w):
            # [NPAD, w] -> [P, NT*w] with tile blocks in finalize order
            return a.reshape(NT, P, w)[fo].transpose(1, 0, 2).reshape(
                P, NT * w).copy()

        m = {
            "xT": xT.astype(nbf),
            "dstloc": dstloc.astype(nbf),
            "dsti": dsti,
            "dsti2": dsti2,
            "ohts": pmf(ohts, 5),
            "ohcl": pmf(ohcl, 2),
            "vmv": pmf(valid.astype(f32)[:, None], 1),
            "vmm": pmf((g_mk * valid)[:, None], 2 - 1),
        }
        for w in range(NW):
            m[f"srcw{w}"] = srcw[w]
            m[f"srcx{w}"] = srcw2[w]
        m.update(consts)
        in_maps.append(m)

    cfg = dict(N=N, D_IN=D_IN, NPC=NPC, NT=NT, NPAD=NPAD, TBL=TBL, WIN=WIN,
               CH=CH, CHW=CHW, KA=KA, KB=KB, binfo=binfo)
    return cfg, in_maps


# ----------------------------------------------------------------------------
# Device graph
# ----------------------------------------------------------------------------

def _build(cfg):
    import os
    STOPAT = int(os.environ.get("STOPAT", "99"))
    N, D_IN = cfg["N"], cfg["D_IN"]
    NT, NPAD, TBL, WIN = cfg["NT"], cfg["NPAD"], cfg["TBL"], cfg["WIN"]
    CH, CHW = cfg["CH"], cfg["CHW"]
    KA, KB = cfg["KA"], cfg["KB"]
    binfo = cfg["binfo"]
    RG = [list(range(NCORES))]

    kbmax = max(int(cv.sum()) * len(tl) for (_, _, tl, cv) in binfo)

    nc = Bacc("TRN2", target_bir_lowering=False, num_devices=NCORES)

    ein = lambda name, shp, dt: nc.dram_tensor(name, shp, dt, kind="ExternalInput")
    xT_d = ein("xT", [D_IN, NPAD], BF16)
    srcw_d = [ein(f"srcw{w}", [P, max(1, int(CHW[w])) * 8], I16) for w in range(NW)]
    srcx_d = [ein(f"srcx{w}", [P, max(1, int(CHW[w])) * 8], I16) for w in range(NW)]
    dstloc_d = ein("dstloc", [P, CH], BF16)
    dsti_d = ein("dsti", [P, CH * 8], I16)
    dsti2_d = ein("dsti2", [P, CH * 8], I16)
    ohts_d = ein("ohts", [P, NT * 5], F32)
    ohcl_d = ein("ohcl", [P, NT * 2], F32)
    vmv_d = ein("vmv", [P, NT], F32)
    vmm_d = ein("vmm", [P, NT], F32)
    wtab1_d = ein("wtab1", [D_IN, 80], BF16)
    wsd2_d = ein("wsd2", [64, 16], BF16)
    wbig_d = ein("wbig", [P, 256], BF16)
    w1cat_d = ein("w1cat", [64, P], BF16)
    b1cat_d = ein("b1cat", [P, 1], F32)
    wcat2_d = ein("wcat2", [P, 8], BF16)
    bcat2_d = ein("bcat2", [8, 1], F32)
    b1r_d = ein("b1r", [P, 64], F32)
    b2r_d = ein("b2r", [P, 64], F32)
    iota_d = ein("iota", [P, P], BF16)
    identf_d = ein("identf", [P, P], F32)
    ident_d = ein("ident", [P, P], BF16)
    ones_d = ein("ones", [P, 1], F32)

    out_d = nc.dram_tensor("out", [1, 1], F32, kind="ExternalOutput")

    tbl1_loc = nc.dram_tensor("tbl1_loc", [NPAD, RW], BF16)
    tbl1_full = nc.dram_tensor("tbl1_full", [TBL, RW], BF16, addr_space="Shared")
    tbl2_loc = nc.dram_tensor("tbl2_loc", [NPAD, RW], BF16)
    tbl2_full = nc.dram_tensor("tbl2_full", [TBL, RW], BF16, addr_space="Shared")
    ar_in = nc.dram_tensor("ar_in", [1, 8], F32)
    ar_out = nc.dram_tensor("ar_out", [1, 8], F32, addr_space="Shared")

    with tile.TileContext(nc) as tc:
        with (
            tc.tile_pool(name="const", bufs=1) as cp,
            tc.tile_pool(name="sbuf", bufs=2) as sp,
            tc.tile_pool(name="stage", bufs=2) as stp,
            tc.tile_pool(name="psum", bufs=2, space="PSUM") as pp,
        ):
            # ---------------- constants to SBUF ----------------
            def ld(t, dram, shape, dt=BF16):
                s = cp.tile(shape, dt, tag=t, name=t)
                nc.sync.dma_start(out=s[: shape[0]], in_=dram[:])
                return s

            wt1a = cp.tile([KA, 80], BF16, tag="wt1a")
            nc.sync.dma_start(out=wt1a[:], in_=wtab1_d[0:KA, :])
            if KB:
                wt1b = cp.tile([max(KB, 32), 80], BF16, tag="wt1b")
                nc.sync.dma_start(out=wt1b[:KB], in_=wtab1_d[KA:D_IN, :])
            wsd2 = ld("wsd2", wsd2_d, [64, 16])
            wbig = ld("wbig", wbig_d, [P, 256])
            w1cat = ld("w1cat", w1cat_d, [64, P])
            b1cat = ld("b1cat", b1cat_d, [P, 1], F32)
            wcat2 = ld("wcat2", wcat2_d, [P, 8])
            bcat2 = ld("bcat2", bcat2_d, [8, 1], F32)
            b1r = ld("b1r", b1r_d, [P, 64], F32)
            b2r = ld("b2r", b2r_d, [P, 64], F32)
            iota = ld("iota", iota_d, [P, P])
            ident = ld("ident", ident_d, [P, P])
            identf = ld("identf", identf_d, [P, P], F32)
            ones = ld("ones", ones_d, [P, 1], F32)
            srcw = [ld(f"srcw{w}", srcw_d[w], [P, max(1, int(CHW[w])) * 8], I16)
                    for w in range(NW)]
            srcx = [ld(f"srcx{w}", srcx_d[w], [P, max(1, int(CHW[w])) * 8], I16)
                    for w in range(NW)]
            dstloc = ld("dstloc", dstloc_d, [P, CH])
            dsti = ld("dsti", dsti_d, [P, CH * 8], I16)
            dsti2 = ld("dsti2", dsti2_d, [P, CH * 8], I16)
            ohts = ld("ohts", ohts_d, [P, NT * 5], F32)
            ohcl = ld("ohcl", ohcl_d, [P, NT * 2], F32)
            vmv = ld("vmv", vmv_d, [P, NT], F32)
            vmm = ld("vmm", vmm_d, [P, NT], F32)

            # SBUF-resident local table caches: [feat(64)|a_src(8)|a_dst(8)]
            # per tile, written by phase A (layer 1) / fin1 (layer 2)
            tc1 = cp.tile([P, NT * 80], BF16, tag="tc1")
            tc2 = cp.tile([P, NT * 80], BF16, tag="tc2")

            acc = cp.tile([P, 4], F32, tag="acc")
            nc.vector.memset(acc[:], 0.0)

            # ---------------- phase A: layer-1 table ----------------
            WG = 7  # tiles per table-write group
            for g0 in range(0, NT, WG):
                gn = min(WG, NT - g0)
                xa = sp.tile([P, WG * P], BF16, tag="xa")
                nc.sync.dma_start(out=xa[:, 0:gn * P],
                                  in_=xT_d[0:KA, g0 * P:(g0 + gn) * P])
                if KB:
                    xb = sp.tile([max(KB, 32), WG * P], BF16, tag="xb")
                    nc.sync.dma_start(out=xb[:KB, 0:gn * P],
                                      in_=xT_d[KA:D_IN, g0 * P:(g0 + gn) * P])
                for ti in range(gn):
                    t = g0 + ti
                    pA = pp.tile([P, 512], F32, tag="agg", bufs=2)
                    if KB:
                        nc.tensor.matmul(pA[:, 0:80], lhsT=xa[:, ti * P:(ti + 1) * P],
                                         rhs=wt1a[:], start=True, stop=False)
                        nc.tensor.matmul(pA[:, 0:80], lhsT=xb[:KB, ti * P:(ti + 1) * P],
                                         rhs=wt1b[:KB], start=False, stop=True)
                    else:
                        nc.tensor.matmul(pA[:, 0:80], lhsT=xa[:, ti * P:(ti + 1) * P],
                                         rhs=wt1a[:], start=True, stop=True)
                    nc.scalar.activation(tc1[:, t * 80:(t + 1) * 80], pA[:, 0:80],
                                         AF.Copy)
                tdst = tbl1_loc[:].rearrange("(t p) w -> p t w", p=P)[:, g0:g0 + gn, 0:80]
                nc.sync.dma_start(
                    out=tdst,
                    in_=tc1[:, g0 * 80:(g0 + gn) * 80].rearrange(
                        "p (t w) -> p t w", w=80))

            if STOPAT >= 1:
                nc.gpsimd.collective_compute(
                    "AllGather", OP.bypass, ins=[tbl1_loc[:]], outs=[tbl1_full[:]],
                    replica_groups=RG,
                )

            # ---------------- edge phases ----------------
            def edge_layer(layer, tbl_full, tbl_loc, dstidx, tcache, finalize):
                WM = 72 if layer == 1 else 520
                FW = 64 if layer == 1 else 512
                srci = srcw if layer == 1 else srcx
                for (c0, cw0, tiles, cvec) in binfo:
                    nb = len(tiles)
                    S = int(cvec.sum())
                    kb = nb * S
                    # window-major run offsets (in chunks) inside batch slabs
                    woff = []
                    o = 0
                    for w in range(NW):
                        woff.append(o)
                        o += nb * int(cvec[w])
                    gm = sp.tile([P, kbmax * RW], BF16, tag="gm")
                    for w in range(NW):
                        cw = int(cvec[w])
                        if cw == 0:
                            continue
                        kbw = nb * cw
                        nc.gpsimd.dma_gather(
                            out_ap=gm[:, woff[w] * RW:(woff[w] + kbw) * RW]
                                .rearrange("p (c e) -> p c e", e=RW),
                            in_ap=tbl_full[w * WIN:(w + 1) * WIN, :],
                            idxs_ap=srci[w][:, int(cw0[w]) * 8:(int(cw0[w]) + kbw) * 8],
                            num_idxs=kbw * P, num_idxs_reg=kbw * P, elem_size=RW,
                            single_packet=False)
                    # per-edge DST rows from the local table (for a_dst)
                    gd = sp.tile([P, kbmax * RW], BF16, tag="gd")
                    nc.gpsimd.dma_gather(
                        out_ap=gd[:, 0:kb * RW].rearrange("p (c e) -> p c e", e=RW),
                        in_ap=tbl_loc[:],
                        idxs_ap=dstidx[:, c0 * 8:(c0 + kb) * 8],
                        num_idxs=kb * P, num_idxs_reg=kb * P, elem_size=RW,
                        single_packet=False)

                    # one-hot [edge, slot] per chunk (window-major dstloc)
                    oh = sp.tile([P, kbmax * P], BF16, tag="oh")
                    nc.vector.tensor_tensor(
                        out=oh[:, 0:kb * P].rearrange("p (c e) -> p c e", e=P),
                        in0=dstloc[:, c0:c0 + kb].unsqueeze(2).to_broadcast(
                            [P, kb, P]),
                        in1=iota[:].unsqueeze(1).to_broadcast([P, kb, P]),
                        op=OP.is_equal,
                    )

                    # alpha / leaky relu / exp / weighted messages: one op per
                    # batch (window-major layout is contiguous)
                    alpha = sp.tile([P, kbmax * 8], F32, tag="alpha")
                    lrel = sp.tile([P, kbmax * 8], F32, tag="lrel")
                    msg = sp.tile([P, kbmax * WM], BF16, tag="msg")
                    g4 = gm[:, 0:kb * RW].rearrange("p (c e) -> p c e", e=RW)
                    gd4 = gd[:, 0:kb * RW].rearrange("p (c e) -> p c e", e=RW)
                    ms3 = msg[:, 0:kb * WM].rearrange("p (c e) -> p c e", e=WM)
                    nc.vector.tensor_tensor(
                        out=alpha[:, 0:kb * 8].rearrange("p (c e) -> p c e", e=8),
                        in0=g4[:, :, 64:72], in1=gd4[:, :, 72:80], op=OP.add)
                    nc.vector.scalar_tensor_tensor(
                        out=lrel[:, 0:kb * 8],
                        in0=alpha[:, 0:kb * 8], scalar=0.2,
                        in1=alpha[:, 0:kb * 8], op0=OP.mult, op1=OP.max)
                    # exp straight into the msg tail (denominator columns)
                    nc.scalar.activation(
                        ms3[:, :, WM - 8:WM],
                        lrel[:, 0:kb * 8].rearrange("p (c e) -> p c e", e=8),
                        AF.Exp)
                    if layer == 1:
                        nc.vector.tensor_tensor(
                            out=ms3[:, :, 0:64].rearrange("p c (h z) -> p c h z", h=8),
                            in0=g4[:, :, 0:64].rearrange("p c (h z) -> p c h z", h=8),
                            in1=ms3[:, :, 64:72].unsqueeze(3).to_broadcast(
                                [P, kb, 8, 8]),
                            op=OP.mult,
                        )
                    else:
                        nc.vector.tensor_tensor(
                            out=ms3[:, :, 0:512].rearrange("p c (h z) -> p c h z", h=8),
                            in0=g4[:, :, 0:64].unsqueeze(2).to_broadcast(
                                [P, kb, 8, 64]),
                            in1=ms3[:, :, 512:520].unsqueeze(3).to_broadcast(
                                [P, kb, 8, 64]),
                            op=OP.mult,
                        )

                    for i_t, t in enumerate(tiles):
                        pz = pp.tile([P, 512], F32, tag="agg", bufs=2, name="pz")
                        pd = (pp.tile([P, 8], F32, tag="den", bufs=1, name="pd")
                              if layer == 2 else None)
                        first = True
                        done = 0
                        for w in range(NW):
                            cw = int(cvec[w])
                            for q in range(cw):
                                jj = woff[w] + i_t * cw + q
                                ohj = oh[:, jj * P:(jj + 1) * P]
                                mj = msg[:, jj * WM:(jj + 1) * WM]
                                done += 1
                                st, fi = first, (done == S)
                                nc.tensor.matmul(
                                    pz[:, 0:FW + (8 if layer == 1 else 0)],
                                    lhsT=ohj,
                                    rhs=mj[:, 0:FW + (8 if layer == 1 else 0)],
                                    start=st, stop=fi)
                                if layer == 2:
                                    nc.tensor.matmul(pd[:], lhsT=ohj,
                                                     rhs=mj[:, 512:520],
                                                     start=st, stop=fi)
                                first = False
                        finalize(t, pz, pd)

            # ---------------- layer-1 finalize: h1, layer-2 table ----------------
            tbl2_stage = {}
            t2_state = {"n": 0}

            def selfloop_ea(tcache, t):
                # ea of the node's own self-loop: exp(lrelu(a_src + a_dst))
                asum = sp.tile([P, 8], F32, tag="asum")
                nc.vector.tensor_tensor(out=asum[:],
                                        in0=tcache[:, t * 80 + 64:t * 80 + 72],
                                        in1=tcache[:, t * 80 + 72:t * 80 + 80],
                                        op=OP.add)
                lrs = sp.tile([P, 8], F32, tag="lrs")
                nc.vector.scalar_tensor_tensor(out=lrs[:], in0=asum[:], scalar=0.2,
                                               in1=asum[:], op0=OP.mult, op1=OP.max)
                eas = sp.tile([P, 8], F32, tag="eas")
                nc.scalar.activation(eas[:], lrs[:], AF.Exp)
                return eas

            def fin1(t, pz, pd):
                eas = selfloop_ea(tc1, t)
                rin = sp.tile([P, 8], F32, tag="rin")
                nc.vector.scalar_tensor_tensor(out=rin[:], in0=pz[:, 64:72],
                                               scalar=1e-16, in1=eas[:],
                                               op0=OP.add, op1=OP.add)
                rcp = sp.tile([P, 8], F32, tag="rcp")
                nc.vector.reciprocal(rcp[:], rin[:])
                num = sp.tile([P, 64], F32, tag="num")
                nc.vector.tensor_tensor(
                    out=num[:].rearrange("p (h c) -> p h c", h=8),
                    in0=tc1[:, t * 80:t * 80 + 64].rearrange("p (h c) -> p h c", h=8),
                    in1=eas[:].unsqueeze(2).to_broadcast([P, 8, 8]),
                    op=OP.mult,
                )
                nc.vector.tensor_add(num[:], num[:], pz[:, 0:64])
                h1f = sp.tile([P, 64], F32, tag="h1f")
                nc.vector.tensor_tensor(
                    out=h1f[:].rearrange("p (h c) -> p h c", h=8),
                    in0=num[:].rearrange("p (h c) -> p h c", h=8),
                    in1=rcp[:].unsqueeze(2).to_broadcast([P, 8, 8]),
                    op=OP.mult,
                )
                n = t2_state["n"]
                ti = n % 7
                if ti == 0:
                    tbl2_stage[0] = stp.tile([P, 7 * RW], BF16, tag="tbl2_w",
                                             name="tbl2w")
                    nc.vector.memset(tbl2_stage[0][:], 0.0)
                trow = tbl2_stage[0]
                nc.vector.tensor_add(trow[:, ti * RW:ti * RW + 64], h1f[:], b1r[:])
                tp = pp.tile([P, P], BF16, tag="tpb", bufs=1)
                nc.tensor.transpose(tp[0:64, :], trow[:, ti * RW:ti * RW + 64], ident[:])
                h1T = sp.tile([64, P], BF16, tag="h1T")
                nc.scalar.activation(h1T[:], tp[0:64, :], AF.Copy)
                pf = pp.tile([P, 16], F32, tag="hp", bufs=1)
                nc.tensor.matmul(pf[:], lhsT=h1T[:], rhs=wsd2[:], start=True, stop=True)
                nc.scalar.activation(trow[:, ti * RW + 64:ti * RW + 80], pf[:], AF.Copy)
                nc.vector.tensor_copy(tc2[:, t * 80:t * 80 + 64],
                                      trow[:, ti * RW:ti * RW + 64])
                nc.vector.tensor_copy(tc2[:, t * 80 + 64:t * 80 + 80], pf[:])
                t2_state["n"] += 1
                if ti == 6 or t2_state["n"] == NT:
                    gn = ti + 1
                    f0 = t2_state["n"] - gn  # finalize-order position of group start
                    tdst = tbl2_loc[:].rearrange("(t p) w -> p t w", p=P)[
                        :, f0:f0 + gn, :]
                    nc.sync.dma_start(
                        out=tdst,
                        in_=trow[:].rearrange("p (t w) -> p t w", w=RW)[:, 0:gn, :])

            if STOPAT >= 2:
                edge_layer(1, tbl1_full, tbl1_loc, dsti, tc1, fin1)

            if STOPAT >= 3:
                nc.gpsimd.collective_compute(
                    "AllGather", OP.bypass, ins=[tbl2_loc[:]], outs=[tbl2_full[:]],
                    replica_groups=RG,
                )

            # ---------------- layer-2 finalize: h2, MLPs, CE ----------------
            GFIN = 14  # tiles per batched-CE group (even: MLP runs on pairs)
            ceall_ts = cp.tile([P, NT], F32, tag="cets")
            ceall_cl = cp.tile([P, NT], F32, tag="cecl")
            f2 = {"n": 0, "h2T2": None, "slabs": {}}

            def fin2(t, pz, pd):
                n = f2["n"]
                f2["n"] += 1
                g = n // GFIN
                if n % GFIN == 0:
                    f2["slabs"][g] = sp.tile([P, GFIN * 8], F32, tag="tlgs",
                                             name="tlgs")
                eas = selfloop_ea(tc2, t)
                rin = sp.tile([P, 8], F32, tag="rin")
                nc.vector.scalar_tensor_tensor(out=rin[:], in0=pd[:], scalar=1e-16,
                                               in1=eas[:], op0=OP.add, op1=OP.add)
                rcp = sp.tile([P, 8], F32, tag="rcp")
                nc.vector.reciprocal(rcp[:], rin[:])
                num = sp.tile([P, 512], F32, tag="num2")
                nc.vector.tensor_tensor(
                    out=num[:].rearrange("p (h c) -> p h c", h=8),
                    in0=tc2[:, t * 80:t * 80 + 64].unsqueeze(1).to_broadcast([P, 8, 64]),
                    in1=eas[:].unsqueeze(2).to_broadcast([P, 8, 64]),
                    op=OP.mult,
                )
                nc.vector.tensor_add(num[:], num[:], pz[:])
                zn = sp.tile([P, 512], BF16, tag="zn")
                nc.vector.tensor_tensor(
                    out=zn[:].rearrange("p (h c) -> p h c", h=8),
                    in0=num[:].rearrange("p (h c) -> p h c", h=8),
                    in1=rcp[:].unsqueeze(2).to_broadcast([P, 8, 64]),
                    op=OP.mult,
                )
                hp = pp.tile([P, 64], F32, tag="hp", bufs=1)
                tpz = pp.tile([P, 4 * P], BF16, tag="tpb4", bufs=1)
                for k in range(4):
                    nc.tensor.transpose(tpz[:, k * P:(k + 1) * P],
                                        zn[:, k * P:(k + 1) * P], ident[:])
                zT = sp.tile([P, 4 * P], BF16, tag="zT")
                nc.scalar.activation(zT[:], tpz[:], AF.Copy)
                for k in range(4):
                    nc.tensor.matmul(hp[:], lhsT=zT[:, k * P:(k + 1) * P],
                                     rhs=wbig[:, k * 64:(k + 1) * 64],
                                     start=(k == 0), stop=(k == 3))
                h2 = sp.tile([P, 64], BF16, tag="h2")
                nc.vector.tensor_add(h2[:], hp[:], b2r[:])
                tph = pp.tile([P, P], BF16, tag="tpb", bufs=1)
                nc.tensor.transpose(tph[0:64, :], h2[:], ident[:])
                if n % 2 == 0:
                    f2["h2T2"] = sp.tile([64, 2 * P], BF16, tag="h2T2", name="h2T2")
                h2T2 = f2["h2T2"]
                nc.scalar.activation(h2T2[:, (n % 2) * P:(n % 2 + 1) * P],
                                     tph[0:64, :], AF.Copy)

                if n % 2 == 1 or n == NT - 1:
                    npair = (n % 2) + 1
                    pw = npair * P
                    pa = pp.tile([P, 2 * P], F32, tag="tp", bufs=1, name="pa")
                    nc.tensor.matmul(pa[:, 0:pw], lhsT=w1cat[:], rhs=h2T2[:, 0:pw],
                                     start=True, stop=True)
                    h12T = sp.tile([P, 2 * P], BF16, tag="h12T")
                    nc.scalar.activation(h12T[:, 0:pw], pa[:, 0:pw], AF.Relu,
                                         bias=b1cat[:, 0:1])
                    lg = pp.tile([8, 2 * P], F32, tag="tp", bufs=1, name="lg")
                    nc.tensor.matmul(lg[0:8, 0:pw], lhsT=wcat2[:], rhs=h12T[:, 0:pw],
                                     start=True, stop=True)
                    lgsm = sp.tile([8, 2 * P], F32, tag="lgsm")
                    nc.scalar.activation(lgsm[0:8, 0:pw], lg[0:8, 0:pw],
                                         AF.Identity, bias=bcat2[0:8, 0:1])
                    ptl = pp.tile([P, 64], F32, tag="hp", bufs=1, name="ptl")
                    for k in range(npair):
                        nc.tensor.matmul(ptl[:, k * 8:(k + 1) * 8],
                                         lhsT=lgsm[0:8, k * P:(k + 1) * P],
                                         rhs=identf[0:8, 0:8], is_transpose=True,
                                         start=True, stop=True)
                    nk0 = n - (npair - 1)
                    gs = f2["slabs"][nk0 // GFIN]
                    col = nk0 % GFIN
                    nc.vector.tensor_copy(gs[:, col * 8:(col + npair) * 8],
                                          ptl[:, 0:npair * 8])

                if n % GFIN == GFIN - 1 or n == NT - 1:
                    g0 = g * GFIN
                    Gp = n - g0 + 1
                    gs = f2["slabs"].pop(g)
                    tl3 = gs[:, 0:Gp * 8].rearrange("p (t e) -> p t e", e=8)
                    ex_ts = sp.tile([P, GFIN * 5], F32, tag="exts")
                    ex_cl = sp.tile([P, GFIN * 2], F32, tag="excl")
                    nc.scalar.activation(
                        ex_ts[:, 0:Gp * 5].rearrange("p (t e) -> p t e", e=5),
                        tl3[:, :, 0:5], AF.Exp)
                    nc.scalar.activation(
                        ex_cl[:, 0:Gp * 2].rearrange("p (t e) -> p t e", e=2),
                        tl3[:, :, 5:7], AF.Exp)
                    s2g = sp.tile([P, 2 * GFIN], F32, tag="s2g")
                    nc.vector.reduce_sum(
                        s2g[:, 0:Gp].rearrange("p (t e) -> p t e", e=1),
                        ex_ts[:, 0:Gp * 5].rearrange("p (t e) -> p t e", e=5),
                        axis=mybir.AxisListType.X)
                    nc.vector.reduce_sum(
                        s2g[:, GFIN:GFIN + Gp].rearrange("p (t e) -> p t e", e=1),
                        ex_cl[:, 0:Gp * 2].rearrange("p (t e) -> p t e", e=2),
                        axis=mybir.AxisListType.X)
                    lse = sp.tile([P, 2 * GFIN], F32, tag="lseg")
                    nc.scalar.activation(lse[:], s2g[:], AF.Ln)
                    pk_ts = sp.tile([P, GFIN * 5], F32, tag="pkts")
                    pk_cl = sp.tile([P, GFIN * 2], F32, tag="pkcl")
                    nc.vector.tensor_tensor(
                        out=pk_ts[:, 0:Gp * 5].rearrange("p (t e) -> p t e", e=5),
                        in0=tl3[:, :, 0:5],
                        in1=ohts[:, g0 * 5:(g0 + Gp) * 5].rearrange(
                            "p (t e) -> p t e", e=5), op=OP.mult)
                    nc.vector.tensor_tensor(
                        out=pk_cl[:, 0:Gp * 2].rearrange("p (t e) -> p t e", e=2),
                        in0=tl3[:, :, 5:7],
                        in1=ohcl[:, g0 * 2:(g0 + Gp) * 2].rearrange(
                            "p (t e) -> p t e", e=2), op=OP.mult)
                    pks = sp.tile([P, 2 * GFIN], F32, tag="pksg")
                    nc.vector.reduce_sum(
                        pks[:, 0:Gp].rearrange("p (t e) -> p t e", e=1),
                        pk_ts[:, 0:Gp * 5].rearrange("p (t e) -> p t e", e=5),
                        axis=mybir.AxisListType.X)
                    nc.vector.reduce_sum(
                        pks[:, GFIN:GFIN + Gp].rearrange("p (t e) -> p t e", e=1),
                        pk_cl[:, 0:Gp * 2].rearrange("p (t e) -> p t e", e=2),
                        axis=mybir.AxisListType.X)
                    ceg = sp.tile([P, 2 * GFIN], F32, tag="ceg")
                    nc.vector.tensor_sub(ceg[:, 0:Gp], lse[:, 0:Gp], pks[:, 0:Gp])
                    nc.vector.tensor_sub(ceg[:, GFIN:GFIN + Gp],
                                         lse[:, GFIN:GFIN + Gp],
                                         pks[:, GFIN:GFIN + Gp])
                    nc.vector.tensor_tensor(out=ceall_ts[:, g0:g0 + Gp],
                                            in0=ceg[:, 0:Gp],
                                            in1=vmv[:, g0:g0 + Gp], op=OP.mult)
                    nc.vector.tensor_tensor(out=ceall_cl[:, g0:g0 + Gp],
                                            in0=ceg[:, GFIN:GFIN + Gp],
                                            in1=vmm[:, g0:g0 + Gp], op=OP.mult)

            if STOPAT >= 4:
                edge_layer(2, tbl2_full, tbl2_loc, dsti2, tc2, fin2)
                nc.vector.reduce_sum(acc[:, 0:1], ceall_ts[:],
                                     axis=mybir.AxisListType.X)
                nc.vector.reduce_sum(acc[:, 1:2], ceall_cl[:],
                                     axis=mybir.AxisListType.X)
                nc.vector.reduce_sum(acc[:, 2:3], vmm[:],
                                     axis=mybir.AxisListType.X)

            # ---------------- final reduction ----------------
            pfin = pp.tile([1, 8], F32, tag="tp", bufs=1)
            nc.tensor.matmul(pfin[0:1, 0:3], lhsT=ones[:], rhs=acc[:, 0:3],
                             start=True, stop=True)
            fin_sb = cp.tile([1, 8], F32, tag="fin")
            nc.vector.memset(fin_sb[:], 0.0)
            nc.scalar.activation(fin_sb[0:1, 0:3], pfin[0:1, 0:3], AF.Copy)
            nc.sync.dma_start(out=ar_in[:], in_=fin_sb[:])
            nc.gpsimd.collective_compute(
                "AllReduce", OP.add, ins=[ar_in[:]], outs=[ar_out[:]],
                replica_groups=RG,
            )
            tot = cp.tile([1, 8], F32, tag="tot")
            nc.sync.dma_start(out=tot[:], in_=ar_out[:])
            rcpm = cp.tile([1, 1], F32, tag="rcpm")
            nc.vector.reciprocal(rcpm[:], tot[:, 2:3])
            lcl = cp.tile([1, 1], F32, tag="lcl")
            nc.vector.tensor_tensor(out=lcl[:], in0=tot[:, 1:2], in1=rcpm[:], op=OP.mult)
            lts = cp.tile([1, 1], F32, tag="lts")
            nc.vector.tensor_scalar_mul(lts[:], tot[:, 0:1], 1.0 / N)
            res = cp.tile([1, 1], F32, tag="res")
            nc.vector.tensor_add(res[:], lcl[:], lts[:])
            nc.sync.dma_start(out=out_d[:], in_=res[:])

    nc.compile()
    return nc


# ----------------------------------------------------------------------------
# Entry points
# ----------------------------------------------------------------------------

def _run(inputs, trace=False):
    cfg, in_maps = _prep(inputs)
    nc = _build(cfg)
    try:
        r = run_bass_kernel_spmd(nc, in_maps, core_ids=list(range(NCORES)), trace=trace)
    except ModuleNotFoundError:
        r = run_bass_kernel_spmd(nc, in_maps, core_ids=list(range(NCORES)), trace=False)
    out = np.asarray(r.results[0]["out"], np.float32).reshape(())
    return out, r


def kernel(**inputs):
    out, _ = _run(inputs, trace=False)
    return out


def _build_null(cfg):
    """Same I/O signature, trivial compute — for dispatch/transfer baseline."""
    N, D_IN = cfg["N"], cfg["D_IN"]
    NT, NPAD, TBL, WIN = cfg["NT"], cfg["NPAD"], cfg["TBL"], cfg["WIN"]
    CH, CHW = cfg["CH"], cfg["CHW"]
    binfo = cfg["binfo"]
    nc = Bacc("TRN2", target_bir_lowering=False, num_devices=NCORES)
    ein = lambda name, shp, dt: nc.dram_tensor(name, shp, dt, kind="ExternalInput")
    xT_d = ein("xT", [D_IN, NPAD], BF16)
    for w in range(NW):
        ein(f"srcw{w}", [P, max(1, int(CHW[w])) * 8], I16)
        ein(f"srcx{w}", [P, max(1, int(CHW[w])) * 8], I16)
    ein("dstloc", [P, CH], BF16)
    ein("dsti", [P, CH * 8], I16)
    ein("dsti2", [P, CH * 8], I16)
    ein("ohts", [P, NT * 5], F32)
    ein("ohcl", [P, NT * 2], F32)
    ein("vmv", [P, NT], F32)
    ein("vmm", [P, NT], F32)
    ein("wtab1", [D_IN, 80], BF16)
    ein("wsd2", [64, 16], BF16)
    ein("wbig", [P, 256], BF16)
    ein("w1cat", [64, P], BF16)
    ein("b1cat", [P, 1], F32)
    ein("wcat2", [P, 8], BF16)
    ein("bcat2", [8, 1], F32)
    ein("b1r", [P, 64], F32)
    ein("b2r", [P, 64], F32)
    ein("iota", [P, P], BF16)
    identf_d = ein("identf", [P, P], F32)
    ein("ident", [P, P], BF16)
    ein("ones", [P, 1], F32)
    out_d = nc.dram_tensor("out", [1, 1], F32, kind="ExternalOutput")
    with tile.TileContext(nc) as tc:
        with tc.tile_pool(name="sp", bufs=1) as sp:
            t = sp.tile([1, 1], F32, tag="t")
            nc.sync.dma_start(out=t[:], in_=identf_d[0:1, 0:1])
            nc.sync.dma_start(out=out_d[:], in_=t[:])
    nc.compile()
    return nc


# revision 7
# speedup vs baseline: 1.1359x; 1.1359x over previous
"""Distributed Bass kernel for nn_AdaGNN (2-layer GAT + MLP heads + CE losses).

Strategy (8 NeuronCores, SPMD):
  - Nodes assigned to 8 cores x NT tiles of 128 by a load-balancing packer
    (equal edge counts per tile). Output is permutation invariant.
  - Per layer: dense per-node transform producing a 128-wide (256B) gather-table
    row [feat(64) | a_src(8) | a_dst(8) | pad] bf16 -> AllGather -> per-edge
    dma_gather of SRC rows (int16 indices; 4 source windows of TBL/4 rows) and
    of DST rows from the LOCAL table (per-edge a_dst without any transpose) ->
    segment softmax via exp (value ranges are small; max-subtraction
    unnecessary) -> weighted segment-sum via one-hot matmuls on TensorE ->
    normalize.
  - Edge chunks of 128 are keyed (tile, window, q) with a per-tile/window
    chunk schedule shared by all cores (SPMD-static); tiles are grouped into
    batches of identical schedule vectors; all per-edge tensors are laid out
    window-major so per-batch vector ops are single instructions.
  - Layer 2 aggregates per-head-weighted 64-dim inputs (512-wide messages) and
    applies the reshuffled W2 (mean over heads folded in) after aggregation.
  - Layer-2 table rows are stored at finalize-order positions so the staged
    7-tile row groups write with one DMA each.
  - MLP heads + masked CE per dst tile; partial sums AllReduced; final scalar
    computed on device.
"""

import math
import numpy as np
import ml_dtypes

import concourse.bass as bass
import concourse.tile as tile
from concourse import mybir
from concourse.bacc import Bacc
from concourse.bass_utils import run_bass_kernel_spmd

BF16 = mybir.dt.bfloat16
F32 = mybir.dt.float32
I16 = mybir.dt.int16
P = 128
NCORES = 8
NW = 4          # gather windows
RW = 128        # table row width (elements, bf16) = 256B
AF = mybir.ActivationFunctionType
OP = mybir.AluOpType

nbf = ml_dtypes.bfloat16


# ----------------------------------------------------------------------------
# Host-side graph preprocessing
# ----------------------------------------------------------------------------

def _wcat2(tsw2, clsw2):
    w = np.zeros((128, 8), np.float32)
    w[0:64, 0:5] = tsw2
    w[64:128, 5:7] = clsw2
    return w


def _prep(inputs, tiles_per_batch=7):
    x = np.asarray(inputs["x"], np.float32)
    ei = np.asarray(inputs["edge_index"], np.int32)
    N, D_IN = x.shape
    NPC = N // NCORES
    NT = math.ceil(NPC / P)
    NPAD = NT * P
    TBL = NCORES * NPAD
    WIN = TBL // NW
    NBINS = NCORES * NT

    # self-loops are handled analytically on-device (diagonal term of the
    # segment softmax); only the regular edges go through the gather path
    src = ei[0]
    dst = ei[1]

    # ---- balanced node -> (core, tile, slot) assignment ----
    import heapq
    deg = np.bincount(dst, minlength=N).astype(np.int64)
    order_n = np.argsort(-deg, kind="stable")
    heap = [(0, b) for b in range(NBINS)]
    heapq.heapify(heap)
    bin_cnt = np.zeros(NBINS, np.int64)
    bin_edges = np.zeros(NBINS, np.int64)
    node_bin = np.zeros(N, np.int32)
    node_slot = np.zeros(N, np.int32)
    for n in order_n:
        while True:
            e, b = heapq.heappop(heap)
            if e == bin_edges[b] and bin_cnt[b] < P:
                break
        node_bin[n] = b
        node_slot[n] = bin_cnt[b]
        bin_cnt[b] += 1
        bin_edges[b] += deg[n]
        if bin_cnt[b] < P:
            heapq.heappush(heap, (int(bin_edges[b]), b))
    node_core = node_bin // NT
    node_tile = node_bin % NT

    rowpos = node_core.astype(np.int64) * NPAD + node_tile * P + node_slot

    core_of = node_core[dst]
    tile_of = node_tile[dst]
    loc_of = node_slot[dst]
    srow = rowpos[src]
    win_of = (srow // WIN).astype(np.int32)

    # per (core, tile, window) counts -> shared schedule
    cnt = np.zeros((NCORES, NT, NW), np.int64)
    np.add.at(cnt, (core_of, tile_of, win_of), 1)
    chs = np.ceil(cnt / P).astype(np.int64).max(axis=0)  # [NT, NW]
    chs[:, 0] = np.maximum(1, chs[:, 0])  # every tile aggregates >= 1 chunk

    # group tiles by schedule vector; build batches of identical structure
    keys = [tuple(chs[t]) for t in range(NT)]
    order_t = sorted(range(NT), key=lambda t: (keys[t], t))
    batches = []  # (tiles, cvec)
    i = 0
    while i < NT:
        j = i
        S_i = int(sum(keys[order_t[i]]))
        while (j < NT and keys[order_t[j]] == keys[order_t[i]]
               and j - i < tiles_per_batch
               and (j - i + 1) * S_i <= 64):
            j += 1
        batches.append(([order_t[k] for k in range(i, j)],
                        np.array(keys[order_t[i]], np.int64)))
        i = j

    # chunk bookkeeping in batch order
    CH = 0
    CHW = np.zeros(NW, np.int64)
    binfo = []  # (c0, cw0[4], tiles, cvec)
    for tiles, cvec in batches:
        binfo.append((CH, CHW.copy(), tiles, cvec))
        CH += int(cvec.sum()) * len(tiles)
        CHW += cvec * len(tiles)
    CH = int(CH)

    # finalize order (the order edge_layer visits tiles, batch-major) and the
    # layer-2 table row permutation: tbl2 rows live at finalize positions
    fo = np.array([t for (_, _, tiles, _) in binfo for t in tiles], np.int64)
    fp = np.zeros(NT, np.int64)
    fp[fo] = np.arange(NT)
    rowpos2 = node_core.astype(np.int64) * NPAD + fp[node_tile] * P + node_slot
    srow2 = rowpos2[src]

    # per-core edge arrays (chunk positions are WINDOW-MAJOR inside batches)
    per_core = []
    for c in range(NCORES):
        sel = core_of == c
        s_row, s_row2 = srow[sel], srow2[sel]
        t_c, l_c, w_c = tile_of[sel], loc_of[sel], win_of[sel]
        srcw = [np.zeros(max(1, int(CHW[w])) * P, np.int16) for w in range(NW)]
        srcw2 = [np.zeros(max(1, int(CHW[w])) * P, np.int16) for w in range(NW)]
        dstloc = np.full((CH, P), -1.0, np.float32)
        for (c0, cw0, tiles, cvec) in binfo:
            nb = len(tiles)
            woff = []
            o = 0
            for w in range(NW):
                woff.append(o)
                o += nb * int(cvec[w])
            for i_t, t in enumerate(tiles):
                for w in range(NW):
                    cw = int(cvec[w])
                    if cw == 0:
                        continue
                    m = (t_c == t) & (w_c == w)
                    k = int(m.sum())
                    assert k <= cw * P, (k, cw)
                    rows_l = (s_row[m] - w * WIN).astype(np.int16)
                    rows_l2 = (s_row2[m] - w * WIN).astype(np.int16)
                    lt = l_c[m]
                    gp = c0 + woff[w] + i_t * cw       # window-major position
                    wp = int(cw0[w]) + i_t * cw        # window-local position
                    j = np.arange(k)
                    srcw[w][(wp + j // P) * P + (j % P)] = rows_l
                    srcw2[w][(wp + j // P) * P + (j % P)] = rows_l2
                    dstloc[gp + j // P, j % P] = lt

        def wrap(ids):
            a = ids.reshape(-1, 16).T.copy()
            return np.tile(a, (8, 1)).astype(np.int16)

        per_core.append((
            [wrap(srcw[w]) for w in range(NW)],
            [wrap(srcw2[w]) for w in range(NW)],
            dstloc.T.copy(),
        ))

    # ----- weights / constants (replicated) -----
    f32 = np.float32
    W1 = np.asarray(inputs["W1"], f32)
    as1 = np.asarray(inputs["att_src1"], f32)
    ad1 = np.asarray(inputs["att_dst1"], f32)
    W1h = W1.reshape(D_IN, 8, 8)
    wtab1 = np.concatenate(
        [W1, np.einsum("khc,hc->kh", W1h, as1), np.einsum("khc,hc->kh", W1h, ad1)], 1
    )  # [D_IN, 80]
    KA = 128 if D_IN > 128 else D_IN
    KB = D_IN - KA

    W2 = np.asarray(inputs["W2"], f32)
    as2 = np.asarray(inputs["att_src2"], f32)
    ad2 = np.asarray(inputs["att_dst2"], f32)
    W2h = W2.reshape(64, 8, 64)
    wsd2 = np.concatenate(
        [np.einsum("khc,hc->kh", W2h, as2), np.einsum("khc,hc->kh", W2h, ad2)], 1
    )  # [64, 16]
    wbig = (W2h.transpose(1, 0, 2).reshape(512, 64) / 8.0)
    wbig_dev = wbig.reshape(4, 128, 64).transpose(1, 0, 2).reshape(128, 256)

    consts = {
        "wtab1": wtab1.astype(nbf),
        "wsd2": wsd2.astype(nbf),
        "wbig": wbig_dev.astype(nbf),
        "w1cat": np.concatenate([np.asarray(inputs["ts_w1"], f32),
                                 np.asarray(inputs["cls_w1"], f32)], 1).astype(nbf),
        "b1cat": np.concatenate([np.asarray(inputs["ts_b1"], f32),
                                 np.asarray(inputs["cls_b1"], f32)]).reshape(P, 1),
        "wcat2": _wcat2(np.asarray(inputs["ts_w2"], f32),
                        np.asarray(inputs["cls_w2"], f32)).astype(nbf),
        "bcat2": np.concatenate([np.asarray(inputs["ts_b2"], f32),
                                 np.asarray(inputs["cls_b2"], f32),
                                 np.zeros(1, f32)]).reshape(8, 1),
        "b1r": np.tile(np.asarray(inputs["b1"], f32)[None, :], (P, 1)),
        "b2r": np.tile(np.asarray(inputs["b2"], f32)[None, :], (P, 1)),
        "iota": np.tile(np.arange(P, dtype=f32)[None, :], (P, 1)).astype(nbf),
        "ident": np.eye(P, dtype=f32).astype(nbf),
        "identf": np.eye(P, dtype=f32),
        "ones": np.ones((P, 1), f32),
    }

    tst = np.asarray(inputs["timestamp_target"], np.int64)
    clt = np.asarray(inputs["node_target"], np.int64)
    msk = np.asarray(inputs["node_mask"]).astype(f32)

    in_maps = []
    pos_in_core = node_tile.astype(np.int64) * P + node_slot
    for c in range(NCORES):
        srcw, srcw2, dstloc = per_core[c]
        mine = np.nonzero(node_core == c)[0]
        pos = pos_in_core[mine]
        xT = np.zeros((D_IN, NPAD), f32)
        xT[:, pos] = x[mine].T
        valid = np.zeros(NPAD, bool)
        valid[pos] = True
        g_ts = np.zeros(NPAD, np.int64)
        g_ts[pos] = tst[mine]
        g_cl = np.zeros(NPAD, np.int64)
        g_cl[pos] = clt[mine]
        g_mk = np.zeros(NPAD, f32)
        g_mk[pos] = msk[mine]
        rows = np.arange(NPAD)
        ohts = np.zeros((NPAD, 5), f32)
        ohts[rows, g_ts] = 1.0
        ohcl = np.zeros((NPAD, 2), f32)
        ohcl[rows, g_cl] = 1.0

        def pmf(a, w):
            # [NPAD, w] -> [P, NT*w] with tile blocks in finalize order
            return a.reshape(NT, P, w)[fo].transpose(1, 0, 2).reshape(
                P, NT * w).copy()

        m = {
            "xT": xT.astype(nbf),
            "dstloc": dstloc.astype(nbf),
            "ohts": pmf(ohts, 5),
            "ohcl": pmf(ohcl, 2),
            "vmv": pmf(valid.astype(f32)[:, None], 1),
            "vmm": pmf((g_mk * valid)[:, None], 2 - 1),
        }
        for w in range(NW):
            m[f"srcw{w}"] = srcw[w]
            m[f"srcx{w}"] = srcw2[w]
        m.update(consts)
        in_maps.append(m)

    cfg = dict(N=N, D_IN=D_IN, NPC=NPC, NT=NT, NPAD=NPAD, TBL=TBL, WIN=WIN,
               CH=CH, CHW=CHW, KA=KA, KB=KB, binfo=binfo)
    return cfg, in_maps


# ----------------------------------------------------------------------------
# Device graph
# ----------------------------------------------------------------------------

def _build(cfg):
    import os
    STOPAT = int(os.environ.get("STOPAT", "99"))
    N, D_IN = cfg["N"], cfg["D_IN"]
    NT, NPAD, TBL, WIN = cfg["NT"], cfg["NPAD"], cfg["TBL"], cfg["WIN"]
    CH, CHW = cfg["CH"], cfg["CHW"]
    KA, KB = cfg["KA"], cfg["KB"]
    binfo = cfg["binfo"]
    RG = [list(range(NCORES))]

    kbmax = max(int(cv.sum()) * len(tl) for (_, _, tl, cv) in binfo)

    nc = Bacc("TRN2", target_bir_lowering=False, num_devices=NCORES)

    ein = lambda name, shp, dt: nc.dram_tensor(name, shp, dt, kind="ExternalInput")
    xT_d = ein("xT", [D_IN, NPAD], BF16)
    srcw_d = [ein(f"srcw{w}", [P, max(1, int(CHW[w])) * 8], I16) for w in range(NW)]
    srcx_d = [ein(f"srcx{w}", [P, max(1, int(CHW[w])) * 8], I16) for w in range(NW)]
    dstloc_d = ein("dstloc", [P, CH], BF16)
    ohts_d = ein("ohts", [P, NT * 5], F32)
    ohcl_d = ein("ohcl", [P, NT * 2], F32)
    vmv_d = ein("vmv", [P, NT], F32)
    vmm_d = ein("vmm", [P, NT], F32)
    wtab1_d = ein("wtab1", [D_IN, 80], BF16)
    wsd2_d = ein("wsd2", [64, 16], BF16)
    wbig_d = ein("wbig", [P, 256], BF16)
    w1cat_d = ein("w1cat", [64, P], BF16)
    b1cat_d = ein("b1cat", [P, 1], F32)
    wcat2_d = ein("wcat2", [P, 8], BF16)
    bcat2_d = ein("bcat2", [8, 1], F32)
    b1r_d = ein("b1r", [P, 64], F32)
    b2r_d = ein("b2r", [P, 64], F32)
    iota_d = ein("iota", [P, P], BF16)
    identf_d = ein("identf", [P, P], F32)
    ident_d = ein("ident", [P, P], BF16)
    ones_d = ein("ones", [P, 1], F32)

    out_d = nc.dram_tensor("out", [1, 1], F32, kind="ExternalOutput")

    tbl1_loc = nc.dram_tensor("tbl1_loc", [NPAD, RW], BF16)
    tbl1_full = nc.dram_tensor("tbl1_full", [TBL, RW], BF16, addr_space="Shared")
    tbl2_loc = nc.dram_tensor("tbl2_loc", [NPAD, RW], BF16)
    tbl2_full = nc.dram_tensor("tbl2_full", [TBL, RW], BF16, addr_space="Shared")
    ar_in = nc.dram_tensor("ar_in", [1, 8], F32)
    ar_out = nc.dram_tensor("ar_out", [1, 8], F32, addr_space="Shared")

    with tile.TileContext(nc) as tc:
        with (
            tc.tile_pool(name="const", bufs=1) as cp,
            tc.tile_pool(name="sbuf", bufs=2) as sp,
            tc.tile_pool(name="stage", bufs=2) as stp,
            tc.tile_pool(name="psum", bufs=2, space="PSUM") as pp,
        ):
            # ---------------- constants to SBUF ----------------
            def ld(t, dram, shape, dt=BF16):
                s = cp.tile(shape, dt, tag=t, name=t)
                nc.sync.dma_start(out=s[: shape[0]], in_=dram[:])
                return s

            wt1a = cp.tile([KA, 80], BF16, tag="wt1a")
            nc.sync.dma_start(out=wt1a[:], in_=wtab1_d[0:KA, :])
            if KB:
                wt1b = cp.tile([max(KB, 32), 80], BF16, tag="wt1b")
                nc.sync.dma_start(out=wt1b[:KB], in_=wtab1_d[KA:D_IN, :])
            wsd2 = ld("wsd2", wsd2_d, [64, 16])
            wbig = ld("wbig", wbig_d, [P, 256])
            w1cat = ld("w1cat", w1cat_d, [64, P])
            b1cat = ld("b1cat", b1cat_d, [P, 1], F32)
            wcat2 = ld("wcat2", wcat2_d, [P, 8])
            bcat2 = ld("bcat2", bcat2_d, [8, 1], F32)
            b1r = ld("b1r", b1r_d, [P, 64], F32)
            b2r = ld("b2r", b2r_d, [P, 64], F32)
            iota = ld("iota", iota_d, [P, P])
            ident = ld("ident", ident_d, [P, P])
            identf = ld("identf", identf_d, [P, P], F32)
            ones = ld("ones", ones_d, [P, 1], F32)
            srcw = [ld(f"srcw{w}", srcw_d[w], [P, max(1, int(CHW[w])) * 8], I16)
                    for w in range(NW)]
            srcx = [ld(f"srcx{w}", srcx_d[w], [P, max(1, int(CHW[w])) * 8], I16)
                    for w in range(NW)]
            dstloc = ld("dstloc", dstloc_d, [P, CH])
            ohts = ld("ohts", ohts_d, [P, NT * 5], F32)
            ohcl = ld("ohcl", ohcl_d, [P, NT * 2], F32)
            vmv = ld("vmv", vmv_d, [P, NT], F32)
            vmm = ld("vmm", vmm_d, [P, NT], F32)

            # SBUF-resident local table caches: [feat(64)|a_src(8)|a_dst(8)]
            # per tile, written by phase A (layer 1) / fin1 (layer 2)
            tc1 = cp.tile([P, NT * 80], BF16, tag="tc1")
            tc2 = cp.tile([P, NT * 80], BF16, tag="tc2")

            acc = cp.tile([P, 4], F32, tag="acc")
            nc.vector.memset(acc[:], 0.0)

            # ---------------- phase A: layer-1 table ----------------
            WG = 7  # tiles per table-write group
            for g0 in range(0, NT, WG):
                gn = min(WG, NT - g0)
                xa = sp.tile([P, WG * P], BF16, tag="xa")
                nc.sync.dma_start(out=xa[:, 0:gn * P],
                                  in_=xT_d[0:KA, g0 * P:(g0 + gn) * P])
                if KB:
                    xb = sp.tile([max(KB, 32), WG * P], BF16, tag="xb")
                    nc.sync.dma_start(out=xb[:KB, 0:gn * P],
                                      in_=xT_d[KA:D_IN, g0 * P:(g0 + gn) * P])
                for ti in range(gn):
                    t = g0 + ti
                    pA = pp.tile([P, 512], F32, tag="agg", bufs=2)
                    if KB:
                        nc.tensor.matmul(pA[:, 0:80], lhsT=xa[:, ti * P:(ti + 1) * P],
                                         rhs=wt1a[:], start=True, stop=False)
                        nc.tensor.matmul(pA[:, 0:80], lhsT=xb[:KB, ti * P:(ti + 1) * P],
                                         rhs=wt1b[:KB], start=False, stop=True)
                    else:
                        nc.tensor.matmul(pA[:, 0:80], lhsT=xa[:, ti * P:(ti + 1) * P],
                                         rhs=wt1a[:], start=True, stop=True)
                    nc.scalar.activation(tc1[:, t * 80:(t + 1) * 80], pA[:, 0:80],
                                         AF.Copy)
                tdst = tbl1_loc[:].rearrange("(t p) w -> p t w", p=P)[:, g0:g0 + gn, 0:80]
                nc.sync.dma_start(
                    out=tdst,
                    in_=tc1[:, g0 * 80:(g0 + gn) * 80].rearrange(
                        "p (t w) -> p t w", w=80))

            if STOPAT >= 1:
                nc.gpsimd.collective_compute(
                    "AllGather", OP.bypass, ins=[tbl1_loc[:]], outs=[tbl1_full[:]],
                    replica_groups=RG,
                )

            # ---------------- edge phases ----------------
            def edge_layer(layer, tbl_full, tcache, finalize):
                WM = 72 if layer == 1 else 520
                FW = 64 if layer == 1 else 512
                srci = srcw if layer == 1 else srcx
                for (c0, cw0, tiles, cvec) in binfo:
                    nb = len(tiles)
                    S = int(cvec.sum())
                    kb = nb * S
                    # window-major run offsets (in chunks) inside batch slabs
                    woff = []
                    o = 0
                    for w in range(NW):
                        woff.append(o)
                        o += nb * int(cvec[w])
                    gm = sp.tile([P, kbmax * RW], BF16, tag="gm")
                    for w in range(NW):
                        cw = int(cvec[w])
                        if cw == 0:
                            continue
                        kbw = nb * cw
                        nc.gpsimd.dma_gather(
                            out_ap=gm[:, woff[w] * RW:(woff[w] + kbw) * RW]
                                .rearrange("p (c e) -> p c e", e=RW),
                            in_ap=tbl_full[w * WIN:(w + 1) * WIN, :],
                            idxs_ap=srci[w][:, int(cw0[w]) * 8:(int(cw0[w]) + kbw) * 8],
                            num_idxs=kbw * P, num_idxs_reg=kbw * P, elem_size=RW,
                            single_packet=False)
                    # one-hot [edge, slot] per chunk (window-major dstloc)
                    oh = sp.tile([P, kbmax * P], BF16, tag="oh")
                    nc.vector.tensor_tensor(
                        out=oh[:, 0:kb * P].rearrange("p (c e) -> p c e", e=P),
                        in0=dstloc[:, c0:c0 + kb].unsqueeze(2).to_broadcast(
                            [P, kb, P]),
                        in1=iota[:].unsqueeze(1).to_broadcast([P, kb, P]),
                        op=OP.is_equal,
                    )

                    # transposed one-hots: PE transposes into a PSUM slab,
                    # two batched PSUM->SBUF copies, then per-chunk 8-col
                    # matmuls against the local tile's a_dst columns
                    HB = 7
                    ohT = sp.tile([P, kbmax * P], BF16, tag="ohT")
                    for h0 in range(0, kb, HB):
                        hn = min(HB, kb - h0)
                        tpb = pp.tile([P, 7 * P], BF16, tag="tpbB", bufs=2)
                        for i in range(hn):
                            nc.tensor.transpose(tpb[:, i * P:(i + 1) * P],
                                                oh[:, (h0 + i) * P:(h0 + i + 1) * P],
                                                ident[:])
                        nc.scalar.activation(ohT[:, h0 * P:(h0 + hn) * P],
                                             tpb[:, 0:hn * P], AF.Copy)
                    adpe = pp.tile([P, kbmax * 8], F32, tag="adpe", bufs=1)
                    for w in range(NW):
                        cw = int(cvec[w])
                        for i_t, t in enumerate(tiles):
                            for q in range(cw):
                                jj = woff[w] + i_t * cw + q
                                nc.tensor.matmul(
                                    adpe[:, jj * 8:(jj + 1) * 8],
                                    lhsT=ohT[:, jj * P:(jj + 1) * P],
                                    rhs=tcache[:, t * 80 + 72:t * 80 + 80],
                                    start=True, stop=True)

                    # alpha / leaky relu / exp / weighted messages: one op per
                    # batch (window-major layout is contiguous)
                    alpha = sp.tile([P, kbmax * 8], F32, tag="alpha")
                    lrel = sp.tile([P, kbmax * 8], F32, tag="lrel")
                    msg = sp.tile([P, kbmax * WM], BF16, tag="msg")
                    g4 = gm[:, 0:kb * RW].rearrange("p (c e) -> p c e", e=RW)
                    ms3 = msg[:, 0:kb * WM].rearrange("p (c e) -> p c e", e=WM)
                    nc.vector.tensor_tensor(
                        out=alpha[:, 0:kb * 8].rearrange("p (c e) -> p c e", e=8),
                        in0=g4[:, :, 64:72],
                        in1=adpe[:, 0:kb * 8].rearrange("p (c e) -> p c e", e=8),
                        op=OP.add)
                    nc.vector.scalar_tensor_tensor(
                        out=lrel[:, 0:kb * 8],
                        in0=alpha[:, 0:kb * 8], scalar=0.2,
                        in1=alpha[:, 0:kb * 8], op0=OP.mult, op1=OP.max)
                    # exp straight into the msg tail (denominator columns)
                    nc.scalar.activation(
                        ms3[:, :, WM - 8:WM],
                        lrel[:, 0:kb * 8].rearrange("p (c e) -> p c e", e=8),
                        AF.Exp)
                    if layer == 1:
                        nc.vector.tensor_tensor(
                            out=ms3[:, :, 0:64].rearrange("p c (h z) -> p c h z", h=8),
                            in0=g4[:, :, 0:64].rearrange("p c (h z) -> p c h z", h=8),
                            in1=ms3[:, :, 64:72].unsqueeze(3).to_broadcast(
                                [P, kb, 8, 8]),
                            op=OP.mult,
                        )
                    else:
                        nc.vector.tensor_tensor(
                            out=ms3[:, :, 0:512].rearrange("p c (h z) -> p c h z", h=8),
                            in0=g4[:, :, 0:64].unsqueeze(2).to_broadcast(
                                [P, kb, 8, 64]),
                            in1=ms3[:, :, 512:520].unsqueeze(3).to_broadcast(
                                [P, kb, 8, 64]),
                            op=OP.mult,
                        )

                    for i_t, t in enumerate(tiles):
                        pz = pp.tile([P, 512], F32, tag="agg", bufs=2, name="pz")
                        pd = (pp.tile([P, 8], F32, tag="den", bufs=1, name="pd")
                              if layer == 2 else None)
                        first = True
                        done = 0
                        for w in range(NW):
                            cw = int(cvec[w])
                            for q in range(cw):
                                jj = woff[w] + i_t * cw + q
                                ohj = oh[:, jj * P:(jj + 1) * P]
                                mj = msg[:, jj * WM:(jj + 1) * WM]
                                done += 1
                                st, fi = first, (done == S)
                                nc.tensor.matmul(
                                    pz[:, 0:FW + (8 if layer == 1 else 0)],
                                    lhsT=ohj,
                                    rhs=mj[:, 0:FW + (8 if layer == 1 else 0)],
                                    start=st, stop=fi)
                                if layer == 2:
                                    nc.tensor.matmul(pd[:], lhsT=ohj,
                                                     rhs=mj[:, 512:520],
                                                     start=st, stop=fi)
                                first = False
                        finalize(t, pz, pd)

            # ---------------- layer-1 finalize: h1, layer-2 table ----------------
            tbl2_stage = {}
            t2_state = {"n": 0}

            def selfloop_ea(tcache, t):
                # ea of the node's own self-loop: exp(lrelu(a_src + a_dst))
                asum = sp.tile([P, 8], F32, tag="asum")
                nc.vector.tensor_tensor(out=asum[:],
                                        in0=tcache[:, t * 80 + 64:t * 80 + 72],
                                        in1=tcache[:, t * 80 + 72:t * 80 + 80],
                                        op=OP.add)
                lrs = sp.tile([P, 8], F32, tag="lrs")
                nc.vector.scalar_tensor_tensor(out=lrs[:], in0=asum[:], scalar=0.2,
                                               in1=asum[:], op0=OP.mult, op1=OP.max)
                eas = sp.tile([P, 8], F32, tag="eas")
                nc.scalar.activation(eas[:], lrs[:], AF.Exp)
                return eas

            def fin1(t, pz, pd):
                eas = selfloop_ea(tc1, t)
                rin = sp.tile([P, 8], F32, tag="rin")
                nc.vector.scalar_tensor_tensor(out=rin[:], in0=pz[:, 64:72],
                                               scalar=1e-16, in1=eas[:],
                                               op0=OP.add, op1=OP.add)
                rcp = sp.tile([P, 8], F32, tag="rcp")
                nc.vector.reciprocal(rcp[:], rin[:])
                num = sp.tile([P, 64], F32, tag="num")
                nc.vector.tensor_tensor(
                    out=num[:].rearrange("p (h c) -> p h c", h=8),
                    in0=tc1[:, t * 80:t * 80 + 64].rearrange("p (h c) -> p h c", h=8),
                    in1=eas[:].unsqueeze(2).to_broadcast([P, 8, 8]),
                    op=OP.mult,
                )
                nc.vector.tensor_add(num[:], num[:], pz[:, 0:64])
                h1f = sp.tile([P, 64], F32, tag="h1f")
                nc.vector.tensor_tensor(
                    out=h1f[:].rearrange("p (h c) -> p h c", h=8),
                    in0=num[:].rearrange("p (h c) -> p h c", h=8),
                    in1=rcp[:].unsqueeze(2).to_broadcast([P, 8, 8]),
                    op=OP.mult,
                )
                n = t2_state["n"]
                ti = n % 7
                if ti == 0:
                    tbl2_stage[0] = stp.tile([P, 7 * RW], BF16, tag="tbl2_w",
                                             name="tbl2w")
                    nc.vector.memset(tbl2_stage[0][:], 0.0)
                trow = tbl2_stage[0]
                nc.vector.tensor_add(trow[:, ti * RW:ti * RW + 64], h1f[:], b1r[:])
                tp = pp.tile([P, 7 * P], BF16, tag="tpbB", bufs=2)
                nc.tensor.transpose(tp[0:64, 0:P], trow[:, ti * RW:ti * RW + 64],
                                    ident[:])
                h1T = sp.tile([64, P], BF16, tag="h1T")
                nc.scalar.activation(h1T[:], tp[0:64, 0:P], AF.Copy)
                pf = pp.tile([P, 16], F32, tag="hp", bufs=1)
                nc.tensor.matmul(pf[:], lhsT=h1T[:], rhs=wsd2[:], start=True, stop=True)
                nc.scalar.activation(trow[:, ti * RW + 64:ti * RW + 80], pf[:], AF.Copy)
                nc.vector.tensor_copy(tc2[:, t * 80:t * 80 + 64],
                                      trow[:, ti * RW:ti * RW + 64])
                nc.vector.tensor_copy(tc2[:, t * 80 + 64:t * 80 + 80], pf[:])
                t2_state["n"] += 1
                if ti == 6 or t2_state["n"] == NT:
                    gn = ti + 1
                    f0 = t2_state["n"] - gn  # finalize-order position of group start
                    tdst = tbl2_loc[:].rearrange("(t p) w -> p t w", p=P)[
                        :, f0:f0 + gn, :]
                    nc.sync.dma_start(
                        out=tdst,
                        in_=trow[:].rearrange("p (t w) -> p t w", w=RW)[:, 0:gn, :])

            if STOPAT >= 2:
                edge_layer(1, tbl1_full, tc1, fin1)

            if STOPAT >= 3:
                nc.gpsimd.collective_compute(
                    "AllGather", OP.bypass, ins=[tbl2_loc[:]], outs=[tbl2_full[:]],
                    replica_groups=RG,
                )

            # ---------------- layer-2 finalize: h2, MLPs, CE ----------------
            GFIN = 14  # tiles per batched-CE group (even: MLP runs on pairs)
            ceall_ts = cp.tile([P, NT], F32, tag="cets")
            ceall_cl = cp.tile([P, NT], F32, tag="cecl")
            f2 = {"n": 0, "h2T2": None, "slabs": {}}

            def fin2(t, pz, pd):
                n = f2["n"]
                f2["n"] += 1
                g = n // GFIN
                if n % GFIN == 0:
                    f2["slabs"][g] = sp.tile([P, GFIN * 8], F32, tag="tlgs",
                                             name="tlgs")
                eas = selfloop_ea(tc2, t)
                rin = sp.tile([P, 8], F32, tag="rin")
                nc.vector.scalar_tensor_tensor(out=rin[:], in0=pd[:], scalar=1e-16,
                                               in1=eas[:], op0=OP.add, op1=OP.add)
                rcp = sp.tile([P, 8], F32, tag="rcp")
                nc.vector.reciprocal(rcp[:], rin[:])
                num = sp.tile([P, 512], F32, tag="num2")
                nc.vector.tensor_tensor(
                    out=num[:].rearrange("p (h c) -> p h c", h=8),
                    in0=tc2[:, t * 80:t * 80 + 64].unsqueeze(1).to_broadcast([P, 8, 64]),
                    in1=eas[:].unsqueeze(2).to_broadcast([P, 8, 64]),
                    op=OP.mult,
                )
                nc.vector.tensor_add(num[:], num[:], pz[:])
                zn = sp.tile([P, 512], BF16, tag="zn")
                nc.vector.tensor_tensor(
                    out=zn[:].rearrange("p (h c) -> p h c", h=8),
                    in0=num[:].rearrange("p (h c) -> p h c", h=8),
                    in1=rcp[:].unsqueeze(2).to_broadcast([P, 8, 64]),
                    op=OP.mult,
                )
                hp = pp.tile([P, 64], F32, tag="hp", bufs=1)
                tpz = pp.tile([P, 7 * P], BF16, tag="tpbB", bufs=2)
                for k in range(4):
                    nc.tensor.transpose(tpz[:, k * P:(k + 1) * P],
                                        zn[:, k * P:(k + 1) * P], ident[:])
                zT = sp.tile([P, 4 * P], BF16, tag="zT")
                nc.scalar.activation(zT[:], tpz[:, 0:4 * P], AF.Copy)
                for k in range(4):
                    nc.tensor.matmul(hp[:], lhsT=zT[:, k * P:(k + 1) * P],
                                     rhs=wbig[:, k * 64:(k + 1) * 64],
                                     start=(k == 0), stop=(k == 3))
                h2 = sp.tile([P, 64], BF16, tag="h2")
                nc.vector.tensor_add(h2[:], hp[:], b2r[:])
                tph = pp.tile([P, 7 * P], BF16, tag="tpbB", bufs=2)
                nc.tensor.transpose(tph[0:64, 0:P], h2[:], ident[:])
                if n % 2 == 0:
                    f2["h2T2"] = sp.tile([64, 2 * P], BF16, tag="h2T2", name="h2T2")
                h2T2 = f2["h2T2"]
                nc.scalar.activation(h2T2[:, (n % 2) * P:(n % 2 + 1) * P],
                                     tph[0:64, 0:P], AF.Copy)

                if n % 2 == 1 or n == NT - 1:
                    npair = (n % 2) + 1
                    pw = npair * P
                    pa = pp.tile([P, 2 * P], F32, tag="tp", bufs=1, name="pa")
                    nc.tensor.matmul(pa[:, 0:pw], lhsT=w1cat[:], rhs=h2T2[:, 0:pw],
                                     start=True, stop=True)
                    h12T = sp.tile([P, 2 * P], BF16, tag="h12T")
                    nc.scalar.activation(h12T[:, 0:pw], pa[:, 0:pw], AF.Relu,
                                         bias=b1cat[:, 0:1])
                    lg = pp.tile([8, 2 * P], F32, tag="tp", bufs=1, name="lg")
                    nc.tensor.matmul(lg[0:8, 0:pw], lhsT=wcat2[:], rhs=h12T[:, 0:pw],
                                     start=True, stop=True)
                    lgsm = sp.tile([8, 2 * P], F32, tag="lgsm")
                    nc.scalar.activation(lgsm[0:8, 0:pw], lg[0:8, 0:pw],
                                         AF.Identity, bias=bcat2[0:8, 0:1])
                    ptl = pp.tile([P, 64], F32, tag="hp", bufs=1, name="ptl")
                    for k in range(npair):
                        nc.tensor.matmul(ptl[:, k * 8:(k + 1) * 8],
                                         lhsT=lgsm[0:8, k * P:(k + 1) * P],
                                         rhs=identf[0:8, 0:8], is_transpose=True,
                                         start=True, stop=True)
                    nk0 = n - (npair - 1)
                    gs = f2["slabs"][nk0 // GFIN]
                    col = nk0 % GFIN
                    nc.vector.tensor_copy(gs[:, col * 8:(col + npair) * 8],
                                          ptl[:, 0:npair * 8])

                if n % GFIN == GFIN - 1 or n == NT - 1:
                    g0 = g * GFIN
                    Gp = n - g0 + 1
                    gs = f2["slabs"].pop(g)
                    tl3 = gs[:, 0:Gp * 8].rearrange("p (t e) -> p t e", e=8)
                    ex_ts = sp.tile([P, GFIN * 5], F32, tag="exts")
                    ex_cl = sp.tile([P, GFIN * 2], F32, tag="excl")
                    nc.scalar.activation(
                        ex_ts[:, 0:Gp * 5].rearrange("p (t e) -> p t e", e=5),
                        tl3[:, :, 0:5], AF.Exp)
                    nc.scalar.activation(
                        ex_cl[:, 0:Gp * 2].rearrange("p (t e) -> p t e", e=2),
                        tl3[:, :, 5:7], AF.Exp)
                    s2g = sp.tile([P, 2 * GFIN], F32, tag="s2g")
                    nc.vector.reduce_sum(
                        s2g[:, 0:Gp].rearrange("p (t e) -> p t e", e=1),
                        ex_ts[:, 0:Gp * 5].rearrange("p (t e) -> p t e", e=5),
                        axis=mybir.AxisListType.X)
                    nc.vector.reduce_sum(
                        s2g[:, GFIN:GFIN + Gp].rearrange("p (t e) -> p t e", e=1),
                        ex_cl[:, 0:Gp * 2].rearrange("p (t e) -> p t e", e=2),
                        axis=mybir.AxisListType.X)
                    lse = sp.tile([P, 2 * GFIN], F32, tag="lseg")
                    nc.scalar.activation(lse[:], s2g[:], AF.Ln)
                    pk_ts = sp.tile([P, GFIN * 5], F32, tag="pkts")
                    pk_cl = sp.tile([P, GFIN * 2], F32, tag="pkcl")
                    nc.vector.tensor_tensor(
                        out=pk_ts[:, 0:Gp * 5].rearrange("p (t e) -> p t e", e=5),
                        in0=tl3[:, :, 0:5],
                        in1=ohts[:, g0 * 5:(g0 + Gp) * 5].rearrange(
                            "p (t e) -> p t e", e=5), op=OP.mult)
                    nc.vector.tensor_tensor(
                        out=pk_cl[:, 0:Gp * 2].rearrange("p (t e) -> p t e", e=2),
                        in0=tl3[:, :, 5:7],
                        in1=ohcl[:, g0 * 2:(g0 + Gp) * 2].rearrange(
                            "p (t e) -> p t e", e=2), op=OP.mult)
                    pks = sp.tile([P, 2 * GFIN], F32, tag="pksg")
                    nc.vector.reduce_sum(
                        pks[:, 0:Gp].rearrange("p (t e) -> p t e", e=1),
                        pk_ts[:, 0:Gp * 5].rearrange("p (t e) -> p t e", e=5),
                        axis=mybir.AxisListType.X)
                    nc.vector.reduce_sum(
                        pks[:, GFIN:GFIN + Gp].rearrange("p (t e) -> p t e", e=1),
                        pk_cl[:, 0:Gp * 2].rearrange("p (t e) -> p t e", e=2),
                        axis=mybir.AxisListType.X)
                    ceg = sp.tile([P, 2 * GFIN], F32, tag="ceg")
                    nc.vector.tensor_sub(ceg[:, 0:Gp], lse[:, 0:Gp], pks[:, 0:Gp])
                    nc.vector.tensor_sub(ceg[:, GFIN:GFIN + Gp],
                                         lse[:, GFIN:GFIN + Gp],
                                         pks[:, GFIN:GFIN + Gp])
                    nc.vector.tensor_tensor(out=ceall_ts[:, g0:g0 + Gp],
                                            in0=ceg[:, 0:Gp],
                                            in1=vmv[:, g0:g0 + Gp], op=OP.mult)
                    nc.vector.tensor_tensor(out=ceall_cl[:, g0:g0 + Gp],
                                            in0=ceg[:, GFIN:GFIN + Gp],
                                            in1=vmm[:, g0:g0 + Gp], op=OP.mult)

            if STOPAT >= 4:
                edge_layer(2, tbl2_full, tc2, fin2)
                nc.vector.reduce_sum(acc[:, 0:1], ceall_ts[:],
                                     axis=mybir.AxisListType.X)
                nc.vector.reduce_sum(acc[:, 1:2], ceall_cl[:],
                                     axis=mybir.AxisListType.X)
                nc.vector.reduce_sum(acc[:, 2:3], vmm[:],
                                     axis=mybir.AxisListType.X)

            # ---------------- final reduction ----------------
            pfin = pp.tile([1, 8], F32, tag="tp", bufs=1)
            nc.tensor.matmul(pfin[0:1, 0:3], lhsT=ones[:], rhs=acc[:, 0:3],
                             start=True, stop=True)
            fin_sb = cp.tile([1, 8], F32, tag="fin")
            nc.vector.memset(fin_sb[:], 0.0)
            nc.scalar.activation(fin_sb[0:1, 0:3], pfin[0:1, 0:3], AF.Copy)
            nc.sync.dma_start(out=ar_in[:], in_=fin_sb[:])
            nc.gpsimd.collective_compute(
                "AllReduce", OP.add, ins=[ar_in[:]], outs=[ar_out[:]],
                replica_groups=RG,
            )
            tot = cp.tile([1, 8], F32, tag="tot")
            nc.sync.dma_start(out=tot[:], in_=ar_out[:])
            rcpm = cp.tile([1, 1], F32, tag="rcpm")
            nc.vector.reciprocal(rcpm[:], tot[:, 2:3])
            lcl = cp.tile([1, 1], F32, tag="lcl")
            nc.vector.tensor_tensor(out=lcl[:], in0=tot[:, 1:2], in1=rcpm[:], op=OP.mult)
            lts = cp.tile([1, 1], F32, tag="lts")
            nc.vector.tensor_scalar_mul(lts[:], tot[:, 0:1], 1.0 / N)
            res = cp.tile([1, 1], F32, tag="res")
            nc.vector.tensor_add(res[:], lcl[:], lts[:])
            nc.sync.dma_start(out=out_d[:], in_=res[:])

    nc.compile()
    return nc


# ----------------------------------------------------------------------------
# Entry points
# ----------------------------------------------------------------------------

def _run(inputs, trace=False):
    cfg, in_maps = _prep(inputs)
    nc = _build(cfg)
    try:
        r = run_bass_kernel_spmd(nc, in_maps, core_ids=list(range(NCORES)), trace=trace)
    except ModuleNotFoundError:
        r = run_bass_kernel_spmd(nc, in_maps, core_ids=list(range(NCORES)), trace=False)
    out = np.asarray(r.results[0]["out"], np.float32).reshape(())
    return out, r


def kernel(**inputs):
    out, _ = _run(inputs, trace=False)
    return out


def _build_null(cfg):
    """Same I/O signature, trivial compute — for dispatch/transfer baseline."""
    N, D_IN = cfg["N"], cfg["D_IN"]
    NT, NPAD, TBL, WIN = cfg["NT"], cfg["NPAD"], cfg["TBL"], cfg["WIN"]
    CH, CHW = cfg["CH"], cfg["CHW"]
    binfo = cfg["binfo"]
    nc = Bacc("TRN2", target_bir_lowering=False, num_devices=NCORES)
    ein = lambda name, shp, dt: nc.dram_tensor(name, shp, dt, kind="ExternalInput")
    xT_d = ein("xT", [D_IN, NPAD], BF16)
    for w in range(NW):
        ein(f"srcw{w}", [P, max(1, int(CHW[w])) * 8], I16)
        ein(f"srcx{w}", [P, max(1, int(CHW[w])) * 8], I16)
    ein("dstloc", [P, CH], BF16)
    ein("ohts", [P, NT * 5], F32)
    ein("ohcl", [P, NT * 2], F32)
    ein("vmv", [P, NT], F32)
    ein("vmm", [P, NT], F32)
    ein("wtab1", [D_IN, 80], BF16)
    ein("wsd2", [64, 16], BF16)
    ein("wbig", [P, 256], BF16)
    ein("w1cat", [64, P], BF16)
    ein("b1cat", [P, 1], F32)
    ein("wcat2", [P, 8], BF16)
    ein("bcat2", [8, 1], F32)
    ein("b1r", [P, 64], F32)
    ein("b2r", [P, 64], F32)
    ein("iota", [P, P], BF16)
    identf_d = ein("identf", [P, P], F32)
    ein("ident", [P, P], BF16)
    ein("ones", [P, 1], F32)
    out_d = nc.dram_tensor("out", [1, 1], F32, kind="ExternalOutput")
    with tile.TileContext(nc) as tc:
        with tc.tile_pool(name="sp", bufs=1) as sp:
            t = sp.tile([1, 1], F32, tag="t")
            nc.sync.dma_start(out=t[:], in_=identf_d[0:1, 0:1])
            nc.sync.dma_start(out=out_d[:], in_=t[:])
    nc.compile()
    return nc


# revision 14
# speedup vs baseline: 1.1570x; 1.0186x over previous
"""Distributed Bass kernel for nn_AdaGNN (2-layer GAT + MLP heads + CE losses).

Strategy (8 NeuronCores, SPMD):
  - Nodes assigned to 8 cores x NT tiles of 128 by a load-balancing packer
    (equal edge counts per tile). Output is permutation invariant. Tiles are
    renumbered into schedule (finalize) order so every per-tile cache slice
    is contiguous and table rows are written sequentially.
  - Per layer: dense per-node transform producing a 128-wide (256B) gather-table
    row [feat(64) | a_src(8) | a_dst(8) | pad] bf16 -> AllGather -> per-edge
    dma_gather (int16 indices; 4 source windows of TBL/4 rows each) ->
    per-edge a_dst via batched PE transposes of the one-hots + 8-col matmuls
    -> segment softmax via exp (value ranges are small; max-subtraction
    unnecessary) -> weighted segment-sum via one-hot matmuls on TensorE ->
    batched per-group normalize.
  - Edge chunks of 128 are keyed (tile, window, q) with a per-tile/window
    chunk schedule shared by all cores (SPMD-static); tiles are grouped into
    batches of identical schedule vectors; all per-edge tensors are laid out
    window-major so per-batch vector ops are single instructions.
  - Layer 2 aggregates per-head-weighted 64-dim inputs (512-wide messages) and
    applies the reshuffled W2 (mean over heads folded in) after aggregation.
  - Finalize runs per batch: per-tile PSUM->SBUF copy, then group-batched
    softmax-normalization / layer-2 table build / MLP heads / CE.
  - Partial CE sums AllReduced; final scalar computed on device.
"""

import math
import numpy as np
import ml_dtypes

import concourse.bass as bass
import concourse.tile as tile
from concourse import mybir
from concourse.bacc import Bacc
from concourse.bass_utils import run_bass_kernel_spmd

BF16 = mybir.dt.bfloat16
F32 = mybir.dt.float32
I16 = mybir.dt.int16
P = 128
NCORES = 8
NW = 4          # gather windows
RW = 128        # table row width (elements, bf16) = 256B
AF = mybir.ActivationFunctionType
OP = mybir.AluOpType

nbf = ml_dtypes.bfloat16


# ----------------------------------------------------------------------------
# Host-side graph preprocessing
# ----------------------------------------------------------------------------

def _wcat2(tsw2, clsw2):
    w = np.zeros((128, 8), np.float32)
    w[0:64, 0:5] = tsw2
    w[64:128, 5:7] = clsw2
    return w


def _prep(inputs, tiles_per_batch=7):
    x = np.asarray(inputs["x"], np.float32)
    ei = np.asarray(inputs["edge_index"], np.int32)
    N, D_IN = x.shape
    NPC = N // NCORES
    NT = math.ceil(NPC / P)
    NPAD = NT * P
    TBL = NCORES * NPAD
    WIN = TBL // NW
    NBINS = NCORES * NT

    # self-loops are handled analytically on-device (diagonal term of the
    # segment softmax); only the regular edges go through the gather path
    src = ei[0]
    dst = ei[1]

    # ---- balanced node -> (core, tile, slot) assignment ----
    import heapq
    deg = np.bincount(dst, minlength=N).astype(np.int64)
    order_n = np.argsort(-deg, kind="stable")
    heap = [(0, b) for b in range(NBINS)]
    heapq.heapify(heap)
    bin_cnt = np.zeros(NBINS, np.int64)
    bin_edges = np.zeros(NBINS, np.int64)
    node_bin = np.zeros(N, np.int32)
    node_slot = np.zeros(N, np.int32)
    for n in order_n:
        while True:
            e, b = heapq.heappop(heap)
            if e == bin_edges[b] and bin_cnt[b] < P:
                break
        node_bin[n] = b
        node_slot[n] = bin_cnt[b]
        bin_cnt[b] += 1
        bin_edges[b] += deg[n]
        if bin_cnt[b] < P:
            heapq.heappush(heap, (int(bin_edges[b]), b))
    node_core = node_bin // NT
    node_tile = node_bin % NT

    # per (core, tile, window) counts -> shared schedule (pre-relabel)
    core_of0 = node_core[dst]
    tile_of0 = node_tile[dst]
    rowpos0 = node_core.astype(np.int64) * NPAD + node_tile * P + node_slot
    win_of = (rowpos0[src] // WIN).astype(np.int32)
    cnt = np.zeros((NCORES, NT, NW), np.int64)
    np.add.at(cnt, (core_of0, tile_of0, win_of), 1)
    chs = np.ceil(cnt / P).astype(np.int64).max(axis=0)  # [NT, NW]
    chs[:, 0] = np.maximum(1, chs[:, 0])  # every tile aggregates >= 1 chunk

    # group tiles by schedule vector; batches of identical structure; then
    # RELABEL tiles so the schedule visits 0,1,2,... consecutively
    keys = [tuple(chs[t]) for t in range(NT)]
    order_t = sorted(range(NT), key=lambda t: (keys[t], t))
    fp = np.zeros(NT, np.int64)
    fp[order_t] = np.arange(NT)
    node_tile = fp[node_tile].astype(np.int32)

    batches = []  # (t0, nb, cvec) over relabeled consecutive tiles
    i = 0
    while i < NT:
        j = i
        S_i = int(sum(keys[order_t[i]]))
        while (j < NT and keys[order_t[j]] == keys[order_t[i]]
               and j - i < tiles_per_batch
               and (j - i + 1) * S_i <= 64):
            j += 1
        batches.append((i, j - i, np.array(keys[order_t[i]], np.int64)))
        i = j

    # chunk bookkeeping in batch order
    CH = 0
    CHW = np.zeros(NW, np.int64)
    binfo = []  # (c0, cw0[4], t0, nb, cvec)
    for t0, nb, cvec in batches:
        binfo.append((CH, CHW.copy(), t0, nb, cvec))
        CH += int(cvec.sum()) * nb
        CHW += cvec * nb
    CH = int(CH)

    rowpos = node_core.astype(np.int64) * NPAD + node_tile * P + node_slot
    core_of = node_core[dst]
    tile_of = node_tile[dst]
    loc_of = node_slot[dst]
    srow = rowpos[src]

    # per-core edge arrays (chunk positions are WINDOW-MAJOR inside batches)
    per_core = []
    for c in range(NCORES):
        sel = core_of == c
        s_row = srow[sel]
        t_c, l_c, w_c = tile_of[sel], loc_of[sel], win_of[sel]
        srcw = [np.zeros(max(1, int(CHW[w])) * P, np.int16) for w in range(NW)]
        dstloc = np.full((CH, P), -1.0, np.float32)
        for (c0, cw0, t0, nb, cvec) in binfo:
            woff = []
            o = 0
            for w in range(NW):
                woff.append(o)
                o += nb * int(cvec[w])
            for i_t in range(nb):
                t = t0 + i_t
                for w in range(NW):
                    cw = int(cvec[w])
                    if cw == 0:
                        continue
                    m = (t_c == t) & (w_c == w)
                    k = int(m.sum())
                    assert k <= cw * P, (k, cw)
                    rows_l = (s_row[m] - w * WIN).astype(np.int16)
                    lt = l_c[m]
                    gp = c0 + woff[w] + i_t * cw       # window-major position
                    wp = int(cw0[w]) + i_t * cw        # window-local position
                    j = np.arange(k)
                    srcw[w][(wp + j // P) * P + (j % P)] = rows_l
                    dstloc[gp + j // P, j % P] = lt

        def wrap(ids):
            a = ids.reshape(-1, 16).T.copy()
            return np.tile(a, (8, 1)).astype(np.int16)

        per_core.append(([wrap(srcw[w]) for w in range(NW)], dstloc.T.copy()))

    # ----- weights / constants (replicated) -----
    f32 = np.float32
    W1 = np.asarray(inputs["W1"], f32)
    as1 = np.asarray(inputs["att_src1"], f32)
    ad1 = np.asarray(inputs["att_dst1"], f32)
    W1h = W1.reshape(D_IN, 8, 8)
    wtab1 = np.concatenate(
        [W1, np.einsum("khc,hc->kh", W1h, as1), np.einsum("khc,hc->kh", W1h, ad1)], 1
    )  # [D_IN, 80]
    KA = 128 if D_IN > 128 else D_IN
    KB = D_IN - KA

    W2 = np.asarray(inputs["W2"], f32)
    as2 = np.asarray(inputs["att_src2"], f32)
    ad2 = np.asarray(inputs["att_dst2"], f32)
    W2h = W2.reshape(64, 8, 64)
    wsd2 = np.concatenate(
        [np.einsum("khc,hc->kh", W2h, as2), np.einsum("khc,hc->kh", W2h, ad2)], 1
    )  # [64, 16]
    wbig = (W2h.transpose(1, 0, 2).reshape(512, 64) / 8.0)
    wbig_dev = wbig.reshape(4, 128, 64).transpose(1, 0, 2).reshape(128, 256)

    consts = {
        "wtab1": wtab1.astype(nbf),
        "wsd2": wsd2.astype(nbf),
        "wbig": wbig_dev.astype(nbf),
        "w1cat": np.concatenate([np.asarray(inputs["ts_w1"], f32),
                                 np.asarray(inputs["cls_w1"], f32)], 1).astype(nbf),
        "b1cat": np.concatenate([np.asarray(inputs["ts_b1"], f32),
                                 np.asarray(inputs["cls_b1"], f32)]).reshape(P, 1),
        "wcat2": _wcat2(np.asarray(inputs["ts_w2"], f32),
                        np.asarray(inputs["cls_w2"], f32)).astype(nbf),
        "bcat2": np.concatenate([np.asarray(inputs["ts_b2"], f32),
                                 np.asarray(inputs["cls_b2"], f32),
                                 np.zeros(1, f32)]).reshape(8, 1),
        "b1r": np.tile(np.asarray(inputs["b1"], f32)[None, :], (P, 1)),
        "b2r": np.tile(np.asarray(inputs["b2"], f32)[None, :], (P, 1)),
        "iota": np.tile(np.arange(P, dtype=f32)[None, :], (P, 1)).astype(nbf),
        "ident": np.eye(P, dtype=f32).astype(nbf),
        "identf": np.eye(P, dtype=f32),
        "ones": np.ones((P, 1), f32),
    }

    tst = np.asarray(inputs["timestamp_target"], np.int64)
    clt = np.asarray(inputs["node_target"], np.int64)
    msk = np.asarray(inputs["node_mask"]).astype(f32)

    in_maps = []
    pos_in_core = node_tile.astype(np.int64) * P + node_slot
    for c in range(NCORES):
        srcw, dstloc = per_core[c]
        mine = np.nonzero(node_core == c)[0]
        pos = pos_in_core[mine]
        xT = np.zeros((D_IN, NPAD), f32)
        xT[:, pos] = x[mine].T
        valid = np.zeros(NPAD, bool)
        valid[pos] = True
        g_ts = np.zeros(NPAD, np.int64)
        g_ts[pos] = tst[mine]
        g_cl = np.zeros(NPAD, np.int64)
        g_cl[pos] = clt[mine]
        g_mk = np.zeros(NPAD, f32)
        g_mk[pos] = msk[mine]
        rows = np.arange(NPAD)
        ohts = np.zeros((NPAD, 5), f32)
        ohts[rows, g_ts] = 1.0
        ohcl = np.zeros((NPAD, 2), f32)
        ohcl[rows, g_cl] = 1.0

        def pmf(a, w):
            # [NPAD, w] -> [P, NT*w] (tiles already in schedule order)
            return a.reshape(NT, P, w).transpose(1, 0, 2).reshape(P, NT * w).copy()

        m = {
            "xT": xT.astype(nbf),
            "dstloc": dstloc.astype(nbf),
            "ohts": pmf(ohts, 5),
            "ohcl": pmf(ohcl, 2),
            "vmv": pmf(valid.astype(f32)[:, None], 1),
            "vmm": pmf((g_mk * valid)[:, None], 2 - 1),
        }
        for w in range(NW):
            m[f"srcw{w}"] = srcw[w]
        m.update(consts)
        in_maps.append(m)

    cfg = dict(N=N, D_IN=D_IN, NPC=NPC, NT=NT, NPAD=NPAD, TBL=TBL, WIN=WIN,
               CH=CH, CHW=CHW, KA=KA, KB=KB, binfo=binfo)
    return cfg, in_maps


# ----------------------------------------------------------------------------
# Device graph
# ----------------------------------------------------------------------------

def _build(cfg):
    import os
    STOPAT = int(os.environ.get("STOPAT", "99"))
    N, D_IN = cfg["N"], cfg["D_IN"]
    NT, NPAD, TBL, WIN = cfg["NT"], cfg["NPAD"], cfg["TBL"], cfg["WIN"]
    CH, CHW = cfg["CH"], cfg["CHW"]
    KA, KB = cfg["KA"], cfg["KB"]
    binfo = cfg["binfo"]
    RG = [list(range(NCORES))]

    kbmax = max(int(cv.sum()) * nb for (_, _, _, nb, cv) in binfo)
    NBMAX = max(nb for (_, _, _, nb, cv) in binfo)

    nc = Bacc("TRN2", target_bir_lowering=False, num_devices=NCORES)

    ein = lambda name, shp, dt: nc.dram_tensor(name, shp, dt, kind="ExternalInput")
    xT_d = ein("xT", [D_IN, NPAD], BF16)
    srcw_d = [ein(f"srcw{w}", [P, max(1, int(CHW[w])) * 8], I16) for w in range(NW)]
    dstloc_d = ein("dstloc", [P, CH], BF16)
    ohts_d = ein("ohts", [P, NT * 5], F32)
    ohcl_d = ein("ohcl", [P, NT * 2], F32)
    vmv_d = ein("vmv", [P, NT], F32)
    vmm_d = ein("vmm", [P, NT], F32)
    wtab1_d = ein("wtab1", [D_IN, 80], BF16)
    wsd2_d = ein("wsd2", [64, 16], BF16)
    wbig_d = ein("wbig", [P, 256], BF16)
    w1cat_d = ein("w1cat", [64, P], BF16)
    b1cat_d = ein("b1cat", [P, 1], F32)
    wcat2_d = ein("wcat2", [P, 8], BF16)
    bcat2_d = ein("bcat2", [8, 1], F32)
    b1r_d = ein("b1r", [P, 64], F32)
    b2r_d = ein("b2r", [P, 64], F32)
    iota_d = ein("iota", [P, P], BF16)
    identf_d = ein("identf", [P, P], F32)
    ident_d = ein("ident", [P, P], BF16)
    ones_d = ein("ones", [P, 1], F32)

    out_d = nc.dram_tensor("out", [1, 1], F32, kind="ExternalOutput")

    tbl1_loc = nc.dram_tensor("tbl1_loc", [NPAD, RW], BF16)
    tbl1_full = nc.dram_tensor("tbl1_full", [TBL, RW], BF16, addr_space="Shared")
    tbl2_loc = nc.dram_tensor("tbl2_loc", [NPAD, RW], BF16)
    tbl2_full = nc.dram_tensor("tbl2_full", [TBL, RW], BF16, addr_space="Shared")
    ar_in = nc.dram_tensor("ar_in", [1, 8], F32)
    ar_out = nc.dram_tensor("ar_out", [1, 8], F32, addr_space="Shared")

    with tile.TileContext(nc) as tc:
        with (
            tc.tile_pool(name="const", bufs=1) as cp,
            tc.tile_pool(name="sbuf", bufs=2) as sp,
            tc.tile_pool(name="stage", bufs=2) as stp,
            tc.tile_pool(name="psum", bufs=2, space="PSUM") as pp,
        ):
            # ---------------- constants to SBUF ----------------
            def ld(t, dram, shape, dt=BF16):
                s = cp.tile(shape, dt, tag=t, name=t)
                nc.sync.dma_start(out=s[: shape[0]], in_=dram[:])
                return s

            wt1a = cp.tile([KA, 80], BF16, tag="wt1a")
            nc.sync.dma_start(out=wt1a[:], in_=wtab1_d[0:KA, :])
            if KB:
                wt1b = cp.tile([max(KB, 32), 80], BF16, tag="wt1b")
                nc.sync.dma_start(out=wt1b[:KB], in_=wtab1_d[KA:D_IN, :])
            wsd2 = ld("wsd2", wsd2_d, [64, 16])
            wbig = ld("wbig", wbig_d, [P, 256])
            w1cat = ld("w1cat", w1cat_d, [64, P])
            b1cat = ld("b1cat", b1cat_d, [P, 1], F32)
            wcat2 = ld("wcat2", wcat2_d, [P, 8])
            bcat2 = ld("bcat2", bcat2_d, [8, 1], F32)
            b1r = ld("b1r", b1r_d, [P, 64], F32)
            b2r = ld("b2r", b2r_d, [P, 64], F32)
            iota = ld("iota", iota_d, [P, P])
            ident = ld("ident", ident_d, [P, P])
            identf = ld("identf", identf_d, [P, P], F32)
            ones = ld("ones", ones_d, [P, 1], F32)
            srcw = [ld(f"srcw{w}", srcw_d[w], [P, max(1, int(CHW[w])) * 8], I16)
                    for w in range(NW)]
            dstloc = ld("dstloc", dstloc_d, [P, CH])
            ohts = ld("ohts", ohts_d, [P, NT * 5], F32)
            ohcl = ld("ohcl", ohcl_d, [P, NT * 2], F32)
            vmv = ld("vmv", vmv_d, [P, NT], F32)
            vmm = ld("vmm", vmm_d, [P, NT], F32)

            # SBUF-resident local table caches: [feat(64)|a_src(8)|a_dst(8)]
            # per tile, written by phase A (layer 1) / fin1 (layer 2)
            tc1 = cp.tile([P, NT * 80], BF16, tag="tc1")
            tc2 = cp.tile([P, NT * 80], BF16, tag="tc2")

            acc = cp.tile([P, 4], F32, tag="acc")
            nc.vector.memset(acc[:], 0.0)

            # ---------------- phase A: layer-1 table ----------------
            WG = 7  # tiles per table-write group
            for g0 in range(0, NT, WG):
                gn = min(WG, NT - g0)
                xa = sp.tile([P, WG * P], BF16, tag="xa")
                nc.sync.dma_start(out=xa[:, 0:gn * P],
                                  in_=xT_d[0:KA, g0 * P:(g0 + gn) * P])
                if KB:
                    xb = sp.tile([max(KB, 32), WG * P], BF16, tag="xb")
                    nc.sync.dma_start(out=xb[:KB, 0:gn * P],
                                      in_=xT_d[KA:D_IN, g0 * P:(g0 + gn) * P])
                for ti in range(gn):
                    t = g0 + ti
                    pA = pp.tile([P, 512], F32, tag="agg", bufs=2)
                    if KB:
                        nc.tensor.matmul(pA[:, 0:80], lhsT=xa[:, ti * P:(ti + 1) * P],
                                         rhs=wt1a[:], start=True, stop=False)
                        nc.tensor.matmul(pA[:, 0:80], lhsT=xb[:KB, ti * P:(ti + 1) * P],
                                         rhs=wt1b[:KB], start=False, stop=True)
                    else:
                        nc.tensor.matmul(pA[:, 0:80], lhsT=xa[:, ti * P:(ti + 1) * P],
                                         rhs=wt1a[:], start=True, stop=True)
                    nc.scalar.activation(tc1[:, t * 80:(t + 1) * 80], pA[:, 0:80],
                                         AF.Copy)
                tdst = tbl1_loc[:].rearrange("(t p) w -> p t w", p=P)[:, g0:g0 + gn, 0:80]
                nc.sync.dma_start(
                    out=tdst,
                    in_=tc1[:, g0 * 80:(g0 + gn) * 80].rearrange(
                        "p (t w) -> p t w", w=80))

            if STOPAT >= 1:
                nc.gpsimd.collective_compute(
                    "AllGather", OP.bypass, ins=[tbl1_loc[:]], outs=[tbl1_full[:]],
                    replica_groups=RG,
                )

            # ---------------- edge phases ----------------
            def edge_layer(layer, tbl_full, tcache, fin_group):
                WM = 72 if layer == 1 else 520
                FW = 64 if layer == 1 else 512
                for (c0, cw0, t0, nb, cvec) in binfo:
                    S = int(cvec.sum())
                    kb = nb * S
                    # window-major run offsets (in chunks) inside batch slabs
                    woff = []
                    o = 0
                    for w in range(NW):
                        woff.append(o)
                        o += nb * int(cvec[w])
                    gm = sp.tile([P, kbmax * RW], BF16, tag="gm")
                    for w in range(NW):
                        cw = int(cvec[w])
                        if cw == 0:
                            continue
                        kbw = nb * cw
                        nc.gpsimd.dma_gather(
                            out_ap=gm[:, woff[w] * RW:(woff[w] + kbw) * RW]
                                .rearrange("p (c e) -> p c e", e=RW),
                            in_ap=tbl_full[w * WIN:(w + 1) * WIN, :],
                            idxs_ap=srcw[w][:, int(cw0[w]) * 8:(int(cw0[w]) + kbw) * 8],
                            num_idxs=kbw * P, num_idxs_reg=kbw * P, elem_size=RW,
                            single_packet=False)

                    # one-hot [edge, slot] per chunk (window-major dstloc)
                    oh = sp.tile([P, kbmax * P], BF16, tag="oh")
                    nc.vector.tensor_tensor(
                        out=oh[:, 0:kb * P].rearrange("p (c e) -> p c e", e=P),
                        in0=dstloc[:, c0:c0 + kb].unsqueeze(2).to_broadcast(
                            [P, kb, P]),
                        in1=iota[:].unsqueeze(1).to_broadcast([P, kb, P]),
                        op=OP.is_equal,
                    )

                    # transposed one-hots: PE transposes into PSUM slabs,
                    # batched PSUM->SBUF copies, then per-chunk 8-col matmuls
                    # against the local tile's a_dst columns
                    ohT = sp.tile([P, kbmax * P], BF16, tag="ohT")
                    for h0 in range(0, kb, 7):
                        hn = min(7, kb - h0)
                        tpb = pp.tile([P, 7 * P], BF16, tag="tpbB", bufs=2)
                        for i in range(hn):
                            nc.tensor.transpose(tpb[:, i * P:(i + 1) * P],
                                                oh[:, (h0 + i) * P:(h0 + i + 1) * P],
                                                ident[:])
                        nc.scalar.activation(ohT[:, h0 * P:(h0 + hn) * P],
                                             tpb[:, 0:hn * P], AF.Copy)
                    adpe = pp.tile([P, kbmax * 8], F32, tag="adpe", bufs=1)
                    for w in range(NW):
                        cw = int(cvec[w])
                        for i_t in range(nb):
                            for q in range(cw):
                                jj = woff[w] + i_t * cw + q
                                nc.tensor.matmul(
                                    adpe[:, jj * 8:(jj + 1) * 8],
                                    lhsT=ohT[:, jj * P:(jj + 1) * P],
                                    rhs=tcache[:, (t0 + i_t) * 80 + 72:
                                               (t0 + i_t) * 80 + 80],
                                    start=True, stop=True)

                    # alpha / leaky relu / exp / weighted messages: one op per
                    # batch (window-major layout is contiguous)
                    alpha = sp.tile([P, kbmax * 8], F32, tag="alpha", bufs=1)
                    msg = sp.tile([P, kbmax * WM], BF16, tag="msg")
                    g4 = gm[:, 0:kb * RW].rearrange("p (c e) -> p c e", e=RW)
                    ms3 = msg[:, 0:kb * WM].rearrange("p (c e) -> p c e", e=WM)
                    nc.vector.tensor_tensor(
                        out=alpha[:, 0:kb * 8].rearrange("p (c e) -> p c e", e=8),
                        in0=g4[:, :, 64:72],
                        in1=adpe[:, 0:kb * 8].rearrange("p (c e) -> p c e", e=8),
                        op=OP.add)
                    nc.vector.scalar_tensor_tensor(
                        out=alpha[:, 0:kb * 8],
                        in0=alpha[:, 0:kb * 8], scalar=0.2,
                        in1=alpha[:, 0:kb * 8], op0=OP.mult, op1=OP.max)
                    # exp straight into the msg tail (denominator columns)
                    nc.scalar.activation(
                        ms3[:, :, WM - 8:WM],
                        alpha[:, 0:kb * 8].rearrange("p (c e) -> p c e", e=8),
                        AF.Exp)
                    if layer == 1:
                        nc.vector.tensor_tensor(
                            out=ms3[:, :, 0:64].rearrange("p c (h z) -> p c h z", h=8),
                            in0=g4[:, :, 0:64].rearrange("p c (h z) -> p c h z", h=8),
                            in1=ms3[:, :, 64:72].unsqueeze(3).to_broadcast(
                                [P, kb, 8, 8]),
                            op=OP.mult,
                        )
                    else:
                        nc.vector.tensor_tensor(
                            out=ms3[:, :, 0:512].rearrange("p c (h z) -> p c h z", h=8),
                            in0=g4[:, :, 0:64].unsqueeze(2).to_broadcast(
                                [P, kb, 8, 64]),
                            in1=ms3[:, :, 512:520].unsqueeze(3).to_broadcast(
                                [P, kb, 8, 64]),
                            op=OP.mult,
                        )

                    # per-tile aggregation + PSUM->SBUF copy into group slabs
                    FWW = FW + (8 if layer == 1 else 0)
                    pzs = sp.tile([P, NBMAX * FWW], F32 if layer == 1 else BF16,
                                  tag=f"pzs{layer}", bufs=1,
                                  name="pzs")
                    pds = (sp.tile([P, NBMAX * 8], F32, tag="pds", bufs=1, name="pds")
                           if layer == 2 else None)
                    pdp = (pp.tile([P, kbmax * 8], F32, tag="adpe", bufs=1,
                                   name="pdp")
                           if layer == 2 else None)
                    for i_t in range(nb):
                        pz = pp.tile([P, 512], F32, tag="agg", bufs=2, name="pz")
                        pd = (pdp[:, i_t * 8:(i_t + 1) * 8]
                              if layer == 2 else None)
                        first = True
                        done = 0
                        for w in range(NW):
                            cw = int(cvec[w])
                            for q in range(cw):
                                jj = woff[w] + i_t * cw + q
                                ohj = oh[:, jj * P:(jj + 1) * P]
                                mj = msg[:, jj * WM:(jj + 1) * WM]
                                done += 1
                                st, fi = first, (done == S)
                                nc.tensor.matmul(pz[:, 0:FWW], lhsT=ohj,
                                                 rhs=mj[:, 0:FWW],
                                                 start=st, stop=fi)
                                if layer == 2:
                                    nc.tensor.matmul(pd[:], lhsT=ohj,
                                                     rhs=mj[:, 512:520],
                                                     start=st, stop=fi)
                                first = False
                        nc.scalar.activation(pzs[:, i_t * FWW:(i_t + 1) * FWW],
                                             pz[:, 0:FWW], AF.Copy)
                        if layer == 2:
                            nc.vector.tensor_copy(pds[:, i_t * 8:(i_t + 1) * 8],
                                                  pd[:])
                    fin_group(t0, nb, pzs, pds)

            # ---------------- group finalizers ----------------
            def selfloop_ea_grp(tcache, t0, nb):
                # ea of each node's own self-loop: exp(lrelu(a_src + a_dst))
                tg = tcache[:, t0 * 80:(t0 + nb) * 80].rearrange(
                    "p (t w) -> p t w", w=80)
                asum = sp.tile([P, NBMAX * 8], F32, tag="asum", bufs=1)
                nc.vector.tensor_tensor(
                    out=asum[:, 0:nb * 8].rearrange("p (t e) -> p t e", e=8),
                    in0=tg[:, :, 64:72], in1=tg[:, :, 72:80], op=OP.add)
                lrs = sp.tile([P, NBMAX * 8], F32, tag="lrs", bufs=1)
                nc.vector.scalar_tensor_tensor(
                    out=lrs[:, 0:nb * 8], in0=asum[:, 0:nb * 8], scalar=0.2,
                    in1=asum[:, 0:nb * 8], op0=OP.mult, op1=OP.max)
                eas = sp.tile([P, NBMAX * 8], F32, tag="eas", bufs=1)
                nc.scalar.activation(eas[:, 0:nb * 8], lrs[:, 0:nb * 8], AF.Exp)
                return eas

            t2_state = {"n": 0}

            def fin1_group(t0, nb, pzs, pds):
                # pzs: [P, nb*72] = [num(64) | denom(8)] per tile
                pz3 = pzs[:, 0:nb * 72].rearrange("p (t e) -> p t e", e=72)
                tg = tc1[:, t0 * 80:(t0 + nb) * 80].rearrange(
                    "p (t w) -> p t w", w=80)
                eas = selfloop_ea_grp(tc1, t0, nb)
                ea3 = eas[:, 0:nb * 8].rearrange("p (t e) -> p t e", e=8)
                rin = sp.tile([P, NBMAX * 8], F32, tag="rin", bufs=1)
                nc.vector.scalar_tensor_tensor(
                    out=rin[:, 0:nb * 8].rearrange("p (t e) -> p t e", e=8),
                    in0=pz3[:, :, 64:72], scalar=1e-16, in1=ea3,
                    op0=OP.add, op1=OP.add)
                rcp = sp.tile([P, NBMAX * 8], F32, tag="rcp", bufs=1)
                nc.vector.reciprocal(rcp[:, 0:nb * 8], rin[:, 0:nb * 8])
                num = sp.tile([P, NBMAX * 64], F32, tag="num", bufs=1)
                nc.vector.tensor_tensor(
                    out=num[:, 0:nb * 64].rearrange("p (t h c) -> p t h c", h=8, c=8),
                    in0=tg[:, :, 0:64].rearrange("p t (h c) -> p t h c", h=8),
                    in1=ea3.unsqueeze(3).to_broadcast([P, nb, 8, 8]),
                    op=OP.mult,
                )
                nc.vector.tensor_tensor(
                    out=num[:, 0:nb * 64].rearrange("p (t e) -> p t e", e=64),
                    in0=num[:, 0:nb * 64].rearrange("p (t e) -> p t e", e=64),
                    in1=pz3[:, :, 0:64], op=OP.add)
                h1f = sp.tile([P, NBMAX * 64], F32, tag="h1f", bufs=1)
                nc.vector.tensor_tensor(
                    out=h1f[:, 0:nb * 64].rearrange("p (t h c) -> p t h c", h=8, c=8),
                    in0=num[:, 0:nb * 64].rearrange("p (t h c) -> p t h c", h=8, c=8),
                    in1=rcp[:, 0:nb * 8].rearrange("p (t e) -> p t e", e=8)
                        .unsqueeze(3).to_broadcast([P, nb, 8, 8]),
                    op=OP.mult,
                )
                # layer-2 table rows: feat = h1f + b1, attn via wsd2
                trow = stp.tile([P, NBMAX * RW], BF16, tag="tbl2_w", name="tbl2w")
                nc.vector.tensor_tensor(
                    out=trow[:, 0:nb * RW].rearrange(
                        "p (t e) -> p t e", e=RW)[:, :, 0:64],
                    in0=h1f[:, 0:nb * 64].rearrange("p (t e) -> p t e", e=64),
                    in1=b1r[:].unsqueeze(1).to_broadcast([P, nb, 64]),
                    op=OP.add)
                # transposes of the nb feature blocks + one batched copy
                tpb = pp.tile([P, 7 * P], BF16, tag="tpbB", bufs=2)
                for i_t in range(nb):
                    nc.tensor.transpose(
                        tpb[0:64, i_t * P:(i_t + 1) * P],
                        trow[:, i_t * RW:i_t * RW + 64], ident[:])
                h1T = sp.tile([64, 7 * P], BF16, tag="h1T", bufs=1)
                nc.scalar.activation(h1T[:, 0:nb * P], tpb[0:64, 0:nb * P], AF.Copy)
                pf = pp.tile([P, NBMAX * 64], F32, tag="hp", bufs=1)
                for i_t in range(nb):
                    nc.tensor.matmul(pf[:, i_t * 16:(i_t + 1) * 16],
                                     lhsT=h1T[:, i_t * P:(i_t + 1) * P],
                                     rhs=wsd2[:], start=True, stop=True)
                nc.scalar.activation(
                    trow[:, 0:nb * RW].rearrange("p (t e) -> p t e", e=RW)[:, :, 64:80],
                    pf[:, 0:nb * 16].rearrange("p (t e) -> p t e", e=16), AF.Copy)
                nc.vector.tensor_copy(
                    tc2[:, t0 * 80:(t0 + nb) * 80].rearrange(
                        "p (t e) -> p t e", e=80),
                    trow[:, 0:nb * RW].rearrange("p (t e) -> p t e", e=RW)[:, :, 0:80])
                tdst = tbl2_loc[:].rearrange("(t p) w -> p t w", p=P)[
                    :, t0:t0 + nb, 0:80]
                nc.sync.dma_start(
                    out=tdst,
                    in_=trow[:, 0:nb * RW].rearrange(
                        "p (t e) -> p t e", e=RW)[:, :, 0:80])
                t2_state["n"] += nb

            if STOPAT >= 2:
                edge_layer(1, tbl1_full, tc1, fin1_group)

            if STOPAT >= 3:
                nc.gpsimd.collective_compute(
                    "AllGather", OP.bypass, ins=[tbl2_loc[:]], outs=[tbl2_full[:]],
                    replica_groups=RG,
                )

            # ---------------- layer-2 finalize: h2, MLPs, CE ----------------
            ceall_ts = cp.tile([P, NT], F32, tag="cets")
            ceall_cl = cp.tile([P, NT], F32, tag="cecl")

            def fin2_group(t0, nb, pzs, pds):
                # pzs: [P, nb*512] per-head numerators; pds: [P, nb*8] denoms
                tg = tc2[:, t0 * 80:(t0 + nb) * 80].rearrange(
                    "p (t w) -> p t w", w=80)
                eas = selfloop_ea_grp(tc2, t0, nb)
                ea3 = eas[:, 0:nb * 8].rearrange("p (t e) -> p t e", e=8)
                rin = sp.tile([P, NBMAX * 8], F32, tag="rin", bufs=1)
                nc.vector.scalar_tensor_tensor(
                    out=rin[:, 0:nb * 8].rearrange("p (t e) -> p t e", e=8),
                    in0=pds[:, 0:nb * 8].rearrange("p (t e) -> p t e", e=8),
                    scalar=1e-16, in1=ea3, op0=OP.add, op1=OP.add)
                rcp = sp.tile([P, NBMAX * 8], F32, tag="rcp", bufs=1)
                nc.vector.reciprocal(rcp[:, 0:nb * 8], rin[:, 0:nb * 8])
                num = sp.tile([P, NBMAX * 512], F32, tag="num2", bufs=1)
                nc.vector.tensor_tensor(
                    out=num[:, 0:nb * 512].rearrange(
                        "p (t h c) -> p t h c", h=8, c=64),
                    in0=tg[:, :, 0:64].unsqueeze(2).to_broadcast([P, nb, 8, 64]),
                    in1=ea3.unsqueeze(3).to_broadcast([P, nb, 8, 64]),
                    op=OP.mult,
                )
                nc.vector.tensor_tensor(
                    out=num[:, 0:nb * 512],
                    in0=num[:, 0:nb * 512], in1=pzs[:, 0:nb * 512], op=OP.add)
                zn = sp.tile([P, NBMAX * 512], BF16, tag="zn", bufs=1)
                nc.vector.tensor_tensor(
                    out=zn[:, 0:nb * 512].rearrange(
                        "p (t h c) -> p t h c", h=8, c=64),
                    in0=num[:, 0:nb * 512].rearrange(
                        "p (t h c) -> p t h c", h=8, c=64),
                    in1=rcp[:, 0:nb * 8].rearrange("p (t e) -> p t e", e=8)
                        .unsqueeze(3).to_broadcast([P, nb, 8, 64]),
                    op=OP.mult,
                )
                # transpose all nb*4 128-blocks; batched copies; wbig matmuls
                nblk = nb * 4
                zT = sp.tile([P, NBMAX * 4 * P], BF16, tag="zT", bufs=1)
                for h0 in range(0, nblk, 7):
                    hn = min(7, nblk - h0)
                    tpb = pp.tile([P, 7 * P], BF16, tag="tpbB", bufs=2)
                    for i in range(hn):
                        nc.tensor.transpose(tpb[:, i * P:(i + 1) * P],
                                            zn[:, (h0 + i) * P:(h0 + i + 1) * P],
                                            ident[:])
                    nc.scalar.activation(zT[:, h0 * P:(h0 + hn) * P],
                                         tpb[:, 0:hn * P], AF.Copy)
                hp = pp.tile([P, NBMAX * 64], F32, tag="hp", bufs=1)
                for i_t in range(nb):
                    for k in range(4):
                        nc.tensor.matmul(
                            hp[:, i_t * 64:(i_t + 1) * 64],
                            lhsT=zT[:, (i_t * 4 + k) * P:(i_t * 4 + k + 1) * P],
                            rhs=wbig[:, k * 64:(k + 1) * 64],
                            start=(k == 0), stop=(k == 3))
                h2 = sp.tile([P, NBMAX * 64], BF16, tag="h2", bufs=1)
                nc.vector.tensor_tensor(
                    out=h2[:, 0:nb * 64].rearrange("p (t e) -> p t e", e=64),
                    in0=hp[:, 0:nb * 64].rearrange("p (t e) -> p t e", e=64),
                    in1=b2r[:].unsqueeze(1).to_broadcast([P, nb, 64]),
                    op=OP.add)
                # transpose h2 per tile; one copy; batched MLP heads
                tpb = pp.tile([P, 7 * P], BF16, tag="tpbB", bufs=2)
                for i_t in range(nb):
                    nc.tensor.transpose(tpb[0:64, i_t * P:(i_t + 1) * P],
                                        h2[:, i_t * 64:(i_t + 1) * 64], ident[:])
                h2T = sp.tile([64, 7 * P], BF16, tag="h2T", bufs=1)
                nc.scalar.activation(h2T[:, 0:nb * P], tpb[0:64, 0:nb * P], AF.Copy)
                pw = nb * P
                pa = pp.tile([P, 7 * P], F32, tag="tp", bufs=1, name="pa")
                for b0 in range(0, nb, 4):
                    bw = min(4, nb - b0) * P
                    nc.tensor.matmul(pa[:, b0 * P:b0 * P + bw], lhsT=w1cat[:],
                                     rhs=h2T[:, b0 * P:b0 * P + bw],
                                     start=True, stop=True)
                h12T = sp.tile([P, 7 * P], BF16, tag="h12T", bufs=1)
                nc.scalar.activation(h12T[:, 0:pw], pa[:, 0:pw], AF.Relu,
                                     bias=b1cat[:, 0:1])
                lgx = pp.tile([P, 7 * P], F32, tag="tp", bufs=1, name="lg")
                lg = lgx[0:8]
                for b0 in range(0, nb, 4):
                    bw = min(4, nb - b0) * P
                    nc.tensor.matmul(lg[0:8, b0 * P:b0 * P + bw], lhsT=wcat2[:],
                                     rhs=h12T[:, b0 * P:b0 * P + bw],
                                     start=True, stop=True)
                lgsm = sp.tile([8, 7 * P], F32, tag="lgsm", bufs=1)
                nc.scalar.activation(lgsm[0:8, 0:pw], lg[0:8, 0:pw],
                                     AF.Identity, bias=bcat2[0:8, 0:1])
                ptlx = pp.tile([P, NBMAX * 64], F32, tag="hp", bufs=1,
                               name="ptl")
                ptl = ptlx[:, 0:NBMAX * 8]
                for i_t in range(nb):
                    nc.tensor.matmul(ptl[:, i_t * 8:(i_t + 1) * 8],
                                     lhsT=lgsm[0:8, i_t * P:(i_t + 1) * P],
                                     rhs=identf[0:8, 0:8], is_transpose=True,
                                     start=True, stop=True)
                # batched CE over the group's tiles
                tl3 = ptl[:, 0:nb * 8].rearrange("p (t e) -> p t e", e=8)
                ex_ts = sp.tile([P, NBMAX * 5], F32, tag="exts", bufs=1)
                ex_cl = sp.tile([P, NBMAX * 2], F32, tag="excl", bufs=1)
                nc.scalar.activation(
                    ex_ts[:, 0:nb * 5].rearrange("p (t e) -> p t e", e=5),
                    tl3[:, :, 0:5], AF.Exp)
                nc.scalar.activation(
                    ex_cl[:, 0:nb * 2].rearrange("p (t e) -> p t e", e=2),
                    tl3[:, :, 5:7], AF.Exp)
                s2g = sp.tile([P, 2 * NBMAX], F32, tag="s2g", bufs=1)
                nc.vector.reduce_sum(
                    s2g[:, 0:nb].rearrange("p (t e) -> p t e", e=1),
                    ex_ts[:, 0:nb * 5].rearrange("p (t e) -> p t e", e=5),
                    axis=mybir.AxisListType.X)
                nc.vector.reduce_sum(
                    s2g[:, NBMAX:NBMAX + nb].rearrange("p (t e) -> p t e", e=1),
                    ex_cl[:, 0:nb * 2].rearrange("p (t e) -> p t e", e=2),
                    axis=mybir.AxisListType.X)
                lse = sp.tile([P, 2 * NBMAX], F32, tag="lseg", bufs=1)
                nc.scalar.activation(lse[:], s2g[:], AF.Ln)
                pk_ts = sp.tile([P, NBMAX * 5], F32, tag="pkts", bufs=1)
                pk_cl = sp.tile([P, NBMAX * 2], F32, tag="pkcl", bufs=1)
                nc.vector.tensor_tensor(
                    out=pk_ts[:, 0:nb * 5].rearrange("p (t e) -> p t e", e=5),
                    in0=tl3[:, :, 0:5],
                    in1=ohts[:, t0 * 5:(t0 + nb) * 5].rearrange(
                        "p (t e) -> p t e", e=5), op=OP.mult)
                nc.vector.tensor_tensor(
                    out=pk_cl[:, 0:nb * 2].rearrange("p (t e) -> p t e", e=2),
                    in0=tl3[:, :, 5:7],
                    in1=ohcl[:, t0 * 2:(t0 + nb) * 2].rearrange(
                        "p (t e) -> p t e", e=2), op=OP.mult)
                pks = sp.tile([P, 2 * NBMAX], F32, tag="pksg", bufs=1)
                nc.vector.reduce_sum(
                    pks[:, 0:nb].rearrange("p (t e) -> p t e", e=1),
                    pk_ts[:, 0:nb * 5].rearrange("p (t e) -> p t e", e=5),
                    axis=mybir.AxisListType.X)
                nc.vector.reduce_sum(
                    pks[:, NBMAX:NBMAX + nb].rearrange("p (t e) -> p t e", e=1),
                    pk_cl[:, 0:nb * 2].rearrange("p (t e) -> p t e", e=2),
                    axis=mybir.AxisListType.X)
                ceg = sp.tile([P, 2 * NBMAX], F32, tag="ceg", bufs=1)
                nc.vector.tensor_sub(ceg[:, 0:nb], lse[:, 0:nb], pks[:, 0:nb])
                nc.vector.tensor_sub(ceg[:, NBMAX:NBMAX + nb],
                                     lse[:, NBMAX:NBMAX + nb],
                                     pks[:, NBMAX:NBMAX + nb])
                nc.vector.tensor_tensor(out=ceall_ts[:, t0:t0 + nb],
                                        in0=ceg[:, 0:nb],
                                        in1=vmv[:, t0:t0 + nb], op=OP.mult)
                nc.vector.tensor_tensor(out=ceall_cl[:, t0:t0 + nb],
                                        in0=ceg[:, NBMAX:NBMAX + nb],
                                        in1=vmm[:, t0:t0 + nb], op=OP.mult)

            if STOPAT >= 4:
                edge_layer(2, tbl2_full, tc2, fin2_group)
                nc.vector.reduce_sum(acc[:, 0:1], ceall_ts[:],
                                     axis=mybir.AxisListType.X)
                nc.vector.reduce_sum(acc[:, 1:2], ceall_cl[:],
                                     axis=mybir.AxisListType.X)
                nc.vector.reduce_sum(acc[:, 2:3], vmm[:],
                                     axis=mybir.AxisListType.X)

            # ---------------- final reduction ----------------
            pfinx = pp.tile([P, 7 * P], F32, tag="tp", bufs=1)
            pfin = pfinx[0:1, 0:8]
            nc.tensor.matmul(pfin[0:1, 0:3], lhsT=ones[:], rhs=acc[:, 0:3],
                             start=True, stop=True)
            fin_sb = cp.tile([1, 8], F32, tag="fin")
            nc.vector.memset(fin_sb[:], 0.0)
            nc.scalar.activation(fin_sb[0:1, 0:3], pfin[0:1, 0:3], AF.Copy)
            nc.sync.dma_start(out=ar_in[:], in_=fin_sb[:])
            nc.gpsimd.collective_compute(
                "AllReduce", OP.add, ins=[ar_in[:]], outs=[ar_out[:]],
                replica_groups=RG,
            )
            tot = cp.tile([1, 8], F32, tag="tot")
            nc.sync.dma_start(out=tot[:], in_=ar_out[:])
            rcpm = cp.tile([1, 1], F32, tag="rcpm")
            nc.vector.reciprocal(rcpm[:], tot[:, 2:3])
            lcl = cp.tile([1, 1], F32, tag="lcl")
            nc.vector.tensor_tensor(out=lcl[:], in0=tot[:, 1:2], in1=rcpm[:], op=OP.mult)
            lts = cp.tile([1, 1], F32, tag="lts")
            nc.vector.tensor_scalar_mul(lts[:], tot[:, 0:1], 1.0 / N)
            res = cp.tile([1, 1], F32, tag="res")
            nc.vector.tensor_add(res[:], lcl[:], lts[:])
            nc.sync.dma_start(out=out_d[:], in_=res[:])

    nc.compile()
    return nc


# ----------------------------------------------------------------------------
# Entry points
# ----------------------------------------------------------------------------

def _run(inputs, trace=False):
    cfg, in_maps = _prep(inputs)
    nc = _build(cfg)
    try:
        r = run_bass_kernel_spmd(nc, in_maps, core_ids=list(range(NCORES)), trace=trace)
    except ModuleNotFoundError:
        r = run_bass_kernel_spmd(nc, in_maps, core_ids=list(range(NCORES)), trace=False)
    out = np.asarray(r.results[0]["out"], np.float32).reshape(())
    return out, r


def kernel(**inputs):
    out, _ = _run(inputs, trace=False)
    return out


def _build_null(cfg):
    """Same I/O signature, trivial compute — for dispatch/transfer baseline."""
    N, D_IN = cfg["N"], cfg["D_IN"]
    NT, NPAD, TBL, WIN = cfg["NT"], cfg["NPAD"], cfg["TBL"], cfg["WIN"]
    CH, CHW = cfg["CH"], cfg["CHW"]
    nc = Bacc("TRN2", target_bir_lowering=False, num_devices=NCORES)
    ein = lambda name, shp, dt: nc.dram_tensor(name, shp, dt, kind="ExternalInput")
    xT_d = ein("xT", [D_IN, NPAD], BF16)
    for w in range(NW):
        ein(f"srcw{w}", [P, max(1, int(CHW[w])) * 8], I16)
    ein("dstloc", [P, CH], BF16)
    ein("ohts", [P, NT * 5], F32)
    ein("ohcl", [P, NT * 2], F32)
    ein("vmv", [P, NT], F32)
    ein("vmm", [P, NT], F32)
    ein("wtab1", [D_IN, 80], BF16)
    ein("wsd2", [64, 16], BF16)
    ein("wbig", [P, 256], BF16)
    ein("w1cat", [64, P], BF16)
    ein("b1cat", [P, 1], F32)
    ein("wcat2", [P, 8], BF16)
    ein("bcat2", [8, 1], F32)
    ein("b1r", [P, 64], F32)
    ein("b2r", [P, 64], F32)
    ein("iota", [P, P], BF16)
    identf_d = ein("identf", [P, P], F32)
    ein("ident", [P, P], BF16)
    ein("ones", [P, 1], F32)
    out_d = nc.dram_tensor("out", [1, 1], F32, kind="ExternalOutput")
    with tile.TileContext(nc) as tc:
        with tc.tile_pool(name="sp", bufs=1) as sp:
            t = sp.tile([1, 1], F32, tag="t")
            nc.sync.dma_start(out=t[:], in_=identf_d[0:1, 0:1])
            nc.sync.dma_start(out=out_d[:], in_=t[:])
    nc.compile()
    return nc
